# revision 12
# baseline (speedup 1.0000x reference)
"""Non-local block (self-attention over 64x64 spatial map) on 8 NeuronCores.

Sharding: data-parallel over batch (B=8 -> 1 image per core). Each core runs
the full N=4096 attention for its image; no collectives.

Per-core layout strategy:
  - theta_x/phi_x stored [O=96, N=4096] fp16; scores computed TRANSPOSED
    (S^T[m, q] chunks) so softmax denominators can be produced by the PE
    itself via an appended ones-column on the PV matmul rhs.
  - exp(S - 45) on ScalarE (constant shift cancels exactly in softmax).
  - PV: lhsT = expS^T slice [128m, 128q], rhs = [g^T | ones] [128m, 97]
    -> y_u [128q, 97] accumulated over 32 m-chunks; col 96 = row sums.
  - normalize with per-partition reciprocal, transpose y back on PE,
    output projection, fp32 residual add, DMA out.
"""

import numpy as np
import ml_dtypes

B, C, O = 8, 192, 96
HH, WW = 64, 64
N = HH * WW           # 4096
NQ = 8                # q-supers of 512
QS = 512
NMC = N // 128        # 32 m-chunks
N_CORES = 8

_CACHE = {}


def _build():
    from contextlib import ExitStack
    import concourse.tile as tile
    from concourse import bacc, mybir
    from concourse.masks import make_identity

    dt = mybir.dt
    AF = mybir.ActivationFunctionType

    nc = bacc.Bacc("TRN2", target_bir_lowering=False, debug=False,
                   num_devices=N_CORES)

    x_d = nc.dram_tensor("x", [C, N], dt.float32, kind="ExternalInput").ap()
    wt_d = {}
    b_d = {}
    for p in ("theta", "phi", "g"):
        wt_d[p] = nc.dram_tensor(f"wt_{p}", [C, O], dt.float16,
                                 kind="ExternalInput").ap()
        b_d[p] = nc.dram_tensor(f"b_{p}", [O, 1], dt.float32,
                                kind="ExternalInput").ap()
    wWT_d = nc.dram_tensor("w_WT", [O, C], dt.float16, kind="ExternalInput").ap()
    bW_d = nc.dram_tensor("b_W", [C, 1], dt.float32, kind="ExternalInput").ap()
    out_d = nc.dram_tensor("out", [C, N], dt.float32, kind="ExternalOutput").ap()

    with tile.TileContext(nc) as tc:
        with ExitStack() as ctx:
            # ---------------- persistent SBUF pools ----------------
            consts = ctx.enter_context(tc.tile_pool(name="consts", bufs=1))
            xpool = ctx.enter_context(tc.tile_pool(name="x", bufs=1))
            acts = ctx.enter_context(tc.tile_pool(name="acts", bufs=1))
            expp = ctx.enter_context(tc.tile_pool(name="exp", bufs=1))
            ypool = ctx.enter_context(tc.tile_pool(name="y", bufs=8))
            ytp = ctx.enter_context(tc.tile_pool(name="yt", bufs=2))
            outp = ctx.enter_context(tc.tile_pool(name="outsb", bufs=3))

            idnbf = consts.tile([128, 128], dt.bfloat16, tag="idnbf")
            make_identity(nc, idnbf[:])

            wt = {}
            bias = {}
            for p in ("theta", "phi", "g"):
                wt[p] = consts.tile([96, 2 * O], dt.float16, tag=f"wt_{p}", name=f"wt_{p}")
                nc.sync.dma_start(wt[p][:, 0:O], wt_d[p][0:96, :])
                nc.sync.dma_start(wt[p][:, O:2 * O], wt_d[p][96:192, :])
                bias[p] = consts.tile([O, 1], dt.float32, tag=f"b_{p}", name=f"b_{p}")
                nc.sync.dma_start(bias[p][:], b_d[p][:])
            wWT = consts.tile([O, C], dt.float16, tag="wWT")
            nc.sync.dma_start(wWT[:], wWT_d[:])
            bW = [consts.tile([96, 1], dt.float32, tag=f"bW{h}", name=f"bW{h}")
                  for h in (0, 1)]
            for h in (0, 1):
                nc.sync.dma_start(bW[h][:], bW_d[96 * h:96 * h + 96, :])

            # x: two row-halves [96, N] fp32 + fp16 copies for matmul
            xf = [xpool.tile([96, N], dt.float32, tag=f"xf{h}", name=f"xf{h}") for h in (0, 1)]
            xh = [xpool.tile([96, N], dt.float16, tag=f"xh{h}", name=f"xh{h}") for h in (0, 1)]
            for h in (0, 1):
                for j in range(NQ):
                    cs = slice(j * QS, (j + 1) * QS)
                    nc.sync.dma_start(xf[h][:, cs], x_d[96 * h:96 * h + 96, cs])
                    nc.vector.tensor_copy(xh[h][:, cs], xf[h][:, cs])

            theta_sb = acts.tile([O, N], dt.float16, tag="theta")
            phi_sb = acts.tile([O, N], dt.float16, tag="phi")
            gt_ones = acts.tile([128, 97 * NMC], dt.bfloat16, tag="gt")
            nc.vector.memset(gt_ones[:], 1.0)

            expS = expp.tile([128, NMC * QS], dt.bfloat16, tag="expS")
            cneg45 = consts.tile([128, 1], dt.float32, tag="cneg45")
            nc.vector.memset(cneg45[:], -45.0)
            ones_f32 = consts.tile([1, 128], dt.float32, tag="ones_f32")
            nc.vector.memset(ones_f32[:], 1.0)

            # ---------------- P1: projections ----------------
            with tc.tile_pool(name="ps_proj", bufs=3, space="PSUM") as ps_proj, \
                 tc.tile_pool(name="ps_gtr", bufs=2, space="PSUM") as ps_gtr, \
                 tc.tile_pool(name="gsb", bufs=1) as gpool:
                g_sb = gpool.tile([O, N], dt.bfloat16, tag="gsb")
                for p, dst in (("theta", theta_sb), ("phi", phi_sb), ("g", g_sb)):
                    for j in range(NQ):
                        cs = slice(j * QS, (j + 1) * QS)
                        ps = ps_proj.tile([O, QS], dt.float32, tag="proj")
                        nc.tensor.matmul(ps[:], wt[p][:, 0:O], xh[0][:, cs],
                                         start=True, stop=False)
                        nc.tensor.matmul(ps[:], wt[p][:, O:2 * O], xh[1][:, cs],
                                         start=False, stop=True)
                        nc.vector.tensor_scalar_add(dst[:, cs], ps[:], bias[p][:])
                # transpose g -> gt_ones chunks [128m, 96] (col 96 stays 1.0)
                for mc in range(NMC):
                    tr = ps_gtr.tile([128, 96], dt.bfloat16, tag="gtr")
                    nc.tensor.transpose(tr[:], g_sb[:, 128 * mc:128 * mc + 128],
                                        idnbf[0:96, 0:96])
                    nc.vector.tensor_copy(
                        gt_ones[:, 97 * mc:97 * mc + 96], tr[:])

            # biased residual: xf += b_W (after xh conversion reads)
            for h in (0, 1):
                nc.vector.tensor_scalar_add(xf[h][:], xf[h][:], bW[h][:])

            # ---------------- P2: attention ----------------
            with tc.tile_pool(name="ps_qk", bufs=2, space="PSUM") as ps_qk, \
                 tc.tile_pool(name="ps_pv", bufs=2, space="PSUM") as ps_pv, \
                 tc.tile_pool(name="ps_li", bufs=1, space="PSUM") as ps_li, \
                 tc.tile_pool(name="ps_out", bufs=1, space="PSUM") as ps_out:
                for qs in range(NQ):
                    qcols = slice(qs * QS, (qs + 1) * QS)
                    ypsum = ps_pv.tile([97, QS], dt.float32, tag="pv",
                                       name=f"pv_{qs}")
                    for pr in range(NMC // 2):
                        ps = ps_qk.tile([128, 2 * QS], dt.float32, tag="qk")
                        for k in (0, 1):
                            mc = 2 * pr + k
                            nc.tensor.matmul(
                                ps[:, k * QS:(k + 1) * QS],
                                phi_sb[:, 128 * mc:128 * mc + 128],
                                theta_sb[:, qcols], start=True, stop=True)
                        nc.scalar.activation(
                            expS[:, 1024 * pr:1024 * pr + 1024], ps[:],
                            AF.Exp, bias=cneg45[:])
                        for k in (0, 1):
                            mc = 2 * pr + k
                            nc.tensor.matmul(
                                ypsum[:], gt_ones[:, 97 * mc:97 * mc + 97],
                                expS[:, 512 * mc:512 * mc + 512],
                                start=(mc == 0), stop=(mc == NMC - 1))
                    # normalization: linv broadcast over partitions via PE
                    linv_sb = ypool.tile([1, QS], dt.float32, tag="linv")
                    nc.vector.reciprocal(linv_sb[:], ypsum[96:97, :])
                    li_ps = ps_li.tile([128, QS], dt.float32, tag="li")
                    nc.tensor.matmul(li_ps[:], ones_f32[:], linv_sb[:],
                                     start=True, stop=True)
                    y_sb = ypool.tile([O, QS], dt.float32, tag="ysb")
                    nc.vector.tensor_copy(y_sb[:], ypsum[0:96, :])
                    yt_sb = ytp.tile([O, QS], dt.float16, tag="yt")
                    nc.vector.tensor_mul(yt_sb[:], y_sb[:], li_ps[0:96, :])
                    for h in (0, 1):
                        pso = ps_out.tile([96, QS], dt.float32, tag="pout")
                        nc.tensor.matmul(pso[:], wWT[:, 96 * h:96 * h + 96],
                                         yt_sb[:], start=True, stop=True)
                        ob = outp.tile([96, QS], dt.float32, tag="ob")
                        nc.vector.tensor_add(ob[:], pso[:], xf[h][:, qcols])
                        nc.sync.dma_start(out_d[96 * h:96 * h + 96, qcols], ob[:])

    nc.compile()
    return nc


def _get_nc():
    if "nc" not in _CACHE:
        _CACHE["nc"] = _build()
    return _CACHE["nc"]


LAST_RESULTS = None


def kernel(x, g_w, g_b, theta_w, theta_b, phi_w, phi_b, W_w, W_b):
    global LAST_RESULTS
    from concourse.bass_utils import run_bass_kernel_spmd

    nc = _get_nc()
    f16 = ml_dtypes.float16 if hasattr(ml_dtypes, "float16") else np.float16

    x = np.asarray(x, dtype=np.float32)
    common = {
        "wt_theta": np.ascontiguousarray(np.asarray(theta_w).T).astype(np.float16),
        "wt_phi": np.ascontiguousarray(np.asarray(phi_w).T).astype(np.float16),
        "wt_g": np.ascontiguousarray(np.asarray(g_w).T).astype(np.float16),
        "w_WT": np.ascontiguousarray(np.asarray(W_w).T).astype(np.float16),
        "b_theta": np.asarray(theta_b, dtype=np.float32).reshape(O, 1),
        "b_phi": np.asarray(phi_b, dtype=np.float32).reshape(O, 1),
        "b_g": np.asarray(g_b, dtype=np.float32).reshape(O, 1),
        "b_W": np.asarray(W_b, dtype=np.float32).reshape(C, 1),
    }
    in_maps = [
        {"x": np.ascontiguousarray(x[b].reshape(C, N)), **common}
        for b in range(B)
    ]
    res = run_bass_kernel_spmd(nc, in_maps, list(range(N_CORES)))
    LAST_RESULTS = res
    out = np.stack([res.results[b]["out"].reshape(C, HH, WW) for b in range(B)])
    return out.astype(np.float32)


# revision 16
# speedup vs baseline: 1.0079x; 1.0079x over previous
"""Non-local block (self-attention over 64x64 spatial map) on 8 NeuronCores.

Sharding: data-parallel over batch (B=8 -> 1 image per core). Each core runs
the full N=4096 attention for its image; no collectives.

Per-core layout strategy:
  - theta_x/phi_x stored [O=96, N=4096] fp16; scores computed TRANSPOSED
    (S^T[m, q] chunks) so softmax denominators can be produced by the PE
    itself via an appended ones-column on the PV matmul rhs.
  - exp(S - 45) on ScalarE (constant shift cancels exactly in softmax).
  - PV: lhsT = expS^T slice [128m, 128q], rhs = [g^T | ones] [128m, 97]
    -> y_u [128q, 97] accumulated over 32 m-chunks; col 96 = row sums.
  - normalize with per-partition reciprocal, transpose y back on PE,
    output projection, fp32 residual add, DMA out.
"""

import numpy as np
import ml_dtypes

B, C, O = 8, 192, 96
HH, WW = 64, 64
N = HH * WW           # 4096
NQ = 8                # q-supers of 512
QS = 512
NMC = N // 128        # 32 m-chunks
N_CORES = 8

_CACHE = {}


def _build():
    from contextlib import ExitStack
    import concourse.tile as tile
    from concourse import bacc, mybir
    from concourse.masks import make_identity

    dt = mybir.dt
    AF = mybir.ActivationFunctionType

    nc = bacc.Bacc("TRN2", target_bir_lowering=False, debug=False,
                   num_devices=N_CORES)

    x_d = nc.dram_tensor("x", [C, N], dt.float32, kind="ExternalInput").ap()
    wt_d = {}
    b_d = {}
    for p in ("theta", "phi", "g"):
        wt_d[p] = nc.dram_tensor(f"wt_{p}", [C, O], dt.float16,
                                 kind="ExternalInput").ap()
        b_d[p] = nc.dram_tensor(f"b_{p}", [O, 1], dt.float32,
                                kind="ExternalInput").ap()
    wWT_d = nc.dram_tensor("w_WT", [O, C], dt.bfloat16, kind="ExternalInput").ap()
    bW_d = nc.dram_tensor("b_W", [C, 1], dt.float32, kind="ExternalInput").ap()
    out_d = nc.dram_tensor("out", [C, N], dt.float32, kind="ExternalOutput").ap()

    with tile.TileContext(nc) as tc:
        with ExitStack() as ctx:
            # ---------------- persistent SBUF pools ----------------
            consts = ctx.enter_context(tc.tile_pool(name="consts", bufs=1))
            xpool = ctx.enter_context(tc.tile_pool(name="x", bufs=1))
            acts = ctx.enter_context(tc.tile_pool(name="acts", bufs=1))
            expp = ctx.enter_context(tc.tile_pool(name="exp", bufs=1))
            ypool = ctx.enter_context(tc.tile_pool(name="y", bufs=8))
            outp = ctx.enter_context(tc.tile_pool(name="outsb", bufs=3))

            idnbf = consts.tile([128, 128], dt.bfloat16, tag="idnbf")
            make_identity(nc, idnbf[:])

            wt = {}
            bias = {}
            for p in ("theta", "phi", "g"):
                wt[p] = consts.tile([96, 2 * O], dt.float16, tag=f"wt_{p}", name=f"wt_{p}")
                nc.sync.dma_start(wt[p][:, 0:O], wt_d[p][0:96, :])
                nc.sync.dma_start(wt[p][:, O:2 * O], wt_d[p][96:192, :])
                bias[p] = consts.tile([O, 1], dt.float32, tag=f"b_{p}", name=f"b_{p}")
                nc.sync.dma_start(bias[p][:], b_d[p][:])
            wWT = consts.tile([O, C], dt.bfloat16, tag="wWT")
            nc.sync.dma_start(wWT[:], wWT_d[:])
            bW = [consts.tile([96, 1], dt.float32, tag=f"bW{h}", name=f"bW{h}")
                  for h in (0, 1)]
            for h in (0, 1):
                nc.sync.dma_start(bW[h][:], bW_d[96 * h:96 * h + 96, :])

            # x: two row-halves [96, N] fp32 + fp16 copies for matmul
            xf = [xpool.tile([96, N], dt.float32, tag=f"xf{h}", name=f"xf{h}") for h in (0, 1)]
            xh = [xpool.tile([96, N], dt.float16, tag=f"xh{h}", name=f"xh{h}") for h in (0, 1)]
            for h in (0, 1):
                for j in range(NQ):
                    cs = slice(j * QS, (j + 1) * QS)
                    nc.sync.dma_start(xf[h][:, cs], x_d[96 * h:96 * h + 96, cs])
                    nc.vector.tensor_copy(xh[h][:, cs], xf[h][:, cs])

            theta_sb = acts.tile([O, N], dt.float16, tag="theta")
            phi_sb = acts.tile([O, N], dt.float16, tag="phi")
            gt_ones = acts.tile([128, 97 * NMC], dt.bfloat16, tag="gt")
            nc.vector.memset(gt_ones[:], 1.0)

            expS = expp.tile([128, NMC * QS], dt.bfloat16, tag="expS")
            cneg45 = consts.tile([128, 1], dt.float32, tag="cneg45")
            nc.vector.memset(cneg45[:], -45.0)
            ones_f32 = consts.tile([1, 128], dt.float32, tag="ones_f32")
            nc.vector.memset(ones_f32[:], 1.0)

            # ---------------- P1: projections ----------------
            with tc.tile_pool(name="ps_proj", bufs=3, space="PSUM") as ps_proj, \
                 tc.tile_pool(name="ps_gtr", bufs=2, space="PSUM") as ps_gtr, \
                 tc.tile_pool(name="gsb", bufs=1) as gpool:
                g_sb = gpool.tile([O, N], dt.bfloat16, tag="gsb")
                for p, dst in (("theta", theta_sb), ("phi", phi_sb), ("g", g_sb)):
                    for j in range(NQ):
                        cs = slice(j * QS, (j + 1) * QS)
                        ps = ps_proj.tile([O, QS], dt.float32, tag="proj")
                        nc.tensor.matmul(ps[:], wt[p][:, 0:O], xh[0][:, cs],
                                         start=True, stop=False)
                        nc.tensor.matmul(ps[:], wt[p][:, O:2 * O], xh[1][:, cs],
                                         start=False, stop=True)
                        nc.vector.tensor_scalar_add(dst[:, cs], ps[:], bias[p][:])
                # transpose g -> gt_ones chunks [128m, 96] (col 96 stays 1.0)
                for mc in range(NMC):
                    tr = ps_gtr.tile([128, 96], dt.bfloat16, tag="gtr")
                    nc.tensor.transpose(tr[:], g_sb[:, 128 * mc:128 * mc + 128],
                                        idnbf[0:96, 0:96])
                    nc.vector.tensor_copy(
                        gt_ones[:, 97 * mc:97 * mc + 96], tr[:])

            # biased residual: xf += b_W (after xh conversion reads)
            for h in (0, 1):
                nc.vector.tensor_scalar_add(xf[h][:], xf[h][:], bW[h][:])

            # ---------------- P2: attention ----------------
            with tc.tile_pool(name="ps_qk", bufs=2, space="PSUM") as ps_qk, \
                 tc.tile_pool(name="ps_pv", bufs=2, space="PSUM") as ps_pv, \
                 tc.tile_pool(name="ps_li", bufs=1, space="PSUM") as ps_li, \
                 tc.tile_pool(name="ps_out", bufs=1, space="PSUM") as ps_out:
                for qs in range(NQ):
                    qcols = slice(qs * QS, (qs + 1) * QS)
                    ypsum = ps_pv.tile([97, QS], dt.float32, tag="pv",
                                       name=f"pv_{qs}")
                    for pr in range(NMC // 2):
                        ps = ps_qk.tile([128, 2 * QS], dt.float32, tag="qk")
                        for k in (0, 1):
                            mc = 2 * pr + k
                            nc.tensor.matmul(
                                ps[:, k * QS:(k + 1) * QS],
                                phi_sb[:, 128 * mc:128 * mc + 128],
                                theta_sb[:, qcols], start=True, stop=True)
                        nc.scalar.activation(
                            expS[:, 1024 * pr:1024 * pr + 1024], ps[:],
                            AF.Exp, bias=cneg45[:])
                        for k in (0, 1):
                            mc = 2 * pr + k
                            nc.tensor.matmul(
                                ypsum[:], gt_ones[:, 97 * mc:97 * mc + 97],
                                expS[:, 512 * mc:512 * mc + 512],
                                start=(mc == 0), stop=(mc == NMC - 1))
                    # evac unnormalized y (bf16 covers its dynamic range);
                    # normalization commutes with the linear out-projection,
                    # so 1/l runs off the critical path.
                    y_bf = ypool.tile([O, QS], dt.bfloat16, tag="ybf")
                    nc.vector.tensor_copy(y_bf[:], ypsum[0:96, :])
                    linv_sb = ypool.tile([1, QS], dt.float32, tag="linv")
                    nc.vector.reciprocal(linv_sb[:], ypsum[96:97, :])
                    li_ps = ps_li.tile([128, QS], dt.float32, tag="li",
                                       name=f"li_{qs}")
                    nc.tensor.matmul(li_ps[:], ones_f32[:], linv_sb[:],
                                     start=True, stop=True)
                    li_sb = ypool.tile([O, QS], dt.float32, tag="lisb")
                    nc.vector.tensor_copy(li_sb[:], li_ps[0:96, :])
                    for h in (0, 1):
                        pso = ps_out.tile([96, QS], dt.float32, tag="pout")
                        nc.tensor.matmul(pso[:], wWT[:, 96 * h:96 * h + 96],
                                         y_bf[:], start=True, stop=True)
                        ob = outp.tile([96, QS], dt.float32, tag="ob")
                        nc.vector.tensor_mul(ob[:], pso[:], li_sb[:])
                        nc.vector.tensor_add(ob[:], ob[:], xf[h][:, qcols])
                        nc.sync.dma_start(out_d[96 * h:96 * h + 96, qcols], ob[:])

    nc.compile()
    return nc


def _get_nc():
    if "nc" not in _CACHE:
        _CACHE["nc"] = _build()
    return _CACHE["nc"]


LAST_RESULTS = None


def kernel(x, g_w, g_b, theta_w, theta_b, phi_w, phi_b, W_w, W_b):
    global LAST_RESULTS
    from concourse.bass_utils import run_bass_kernel_spmd

    nc = _get_nc()
    f16 = ml_dtypes.float16 if hasattr(ml_dtypes, "float16") else np.float16

    x = np.asarray(x, dtype=np.float32)
    common = {
        "wt_theta": np.ascontiguousarray(np.asarray(theta_w).T).astype(np.float16),
        "wt_phi": np.ascontiguousarray(np.asarray(phi_w).T).astype(np.float16),
        "wt_g": np.ascontiguousarray(np.asarray(g_w).T).astype(np.float16),
        "w_WT": np.ascontiguousarray(np.asarray(W_w).T).astype(ml_dtypes.bfloat16),
        "b_theta": np.asarray(theta_b, dtype=np.float32).reshape(O, 1),
        "b_phi": np.asarray(phi_b, dtype=np.float32).reshape(O, 1),
        "b_g": np.asarray(g_b, dtype=np.float32).reshape(O, 1),
        "b_W": np.asarray(W_b, dtype=np.float32).reshape(C, 1),
    }
    in_maps = [
        {"x": np.ascontiguousarray(x[b].reshape(C, N)), **common}
        for b in range(B)
    ]
    res = run_bass_kernel_spmd(nc, in_maps, list(range(N_CORES)))
    LAST_RESULTS = res
    out = np.stack([res.results[b]["out"].reshape(C, HH, WW) for b in range(B)])
    return out.astype(np.float32)


# revision 20
# speedup vs baseline: 1.1876x; 1.1783x over previous
"""Non-local block (self-attention over 64x64 spatial map) on 8 NeuronCores.

Sharding: data-parallel over batch (B=8 -> 1 image per core). Each core runs
the full N=4096 attention for its image; no collectives.

Per-core layout strategy:
  - theta_x/phi_x stored [O=96, N=4096] fp16; scores computed TRANSPOSED
    (S^T[m, q] chunks) so softmax denominators come out of the PE itself via
    an appended ones-column on the PV lhsT.
  - exp(S - 45) on ScalarE in [128, 1024] chunks (constant shift cancels
    exactly in softmax; bigger chunks amortize the ACT access bubble).
  - PV: lhsT = [g^T | ones] [128m, 97], rhs = expS^T [128m, 512q]
    -> y_u [97, 512] accumulated over 32 m-chunks; row 96 = softmax sums.
  - normalization commutes with the linear out-projection: 1/l is broadcast
    across partitions with a single-pass float32r PE outer product, applied
    after W^T y_u, off the critical path (tails software-pipelined one
    q-super behind the matmul stream).
"""

import numpy as np
import ml_dtypes

B, C, O = 8, 192, 96
HH, WW = 64, 64
N = HH * WW           # 4096
NQ = 8                # q-supers of 512
QS = 512
NMC = N // 128        # 32 m-chunks
N_CORES = 8

_CACHE = {}


def _build():
    from contextlib import ExitStack
    import concourse.tile as tile
    from concourse import bacc, mybir
    from concourse.masks import make_identity

    dt = mybir.dt
    AF = mybir.ActivationFunctionType

    nc = bacc.Bacc("TRN2", target_bir_lowering=False, debug=False,
                   num_devices=N_CORES)

    x_d = nc.dram_tensor("x", [C, N], dt.float32, kind="ExternalInput").ap()
    xh_d = nc.dram_tensor("xh", [C, N], dt.float16, kind="ExternalInput").ap()
    wt_d = {}
    b_d = {}
    for p in ("theta", "phi", "g"):
        wt_d[p] = nc.dram_tensor(f"wt_{p}", [C, O], dt.float16,
                                 kind="ExternalInput").ap()
        b_d[p] = nc.dram_tensor(f"b_{p}", [O, 1], dt.float32,
                                kind="ExternalInput").ap()
    wWT_d = nc.dram_tensor("w_WT", [O, C], dt.bfloat16, kind="ExternalInput").ap()
    bW_d = nc.dram_tensor("b_W", [C, 1], dt.float32, kind="ExternalInput").ap()
    out_d = nc.dram_tensor("out", [C, N], dt.float32, kind="ExternalOutput").ap()

    with tile.TileContext(nc) as tc:
        with ExitStack() as ctx:
            # ---------------- SBUF pools ----------------
            consts = ctx.enter_context(tc.tile_pool(name="consts", bufs=1))
            xpool = ctx.enter_context(tc.tile_pool(name="x", bufs=1))
            acts = ctx.enter_context(tc.tile_pool(name="acts", bufs=1))
            expp = ctx.enter_context(tc.tile_pool(name="exp", bufs=1))
            ypool = ctx.enter_context(tc.tile_pool(name="y", bufs=3))
            outp = ctx.enter_context(tc.tile_pool(name="outsb", bufs=3))
            # ---------------- PSUM pools (shared by all phases) ----------
            ps_qk = ctx.enter_context(
                tc.tile_pool(name="ps_qk", bufs=2, space="PSUM"))
            ps_pv = ctx.enter_context(
                tc.tile_pool(name="ps_pv", bufs=2, space="PSUM"))
            ps_out = ctx.enter_context(
                tc.tile_pool(name="ps_out", bufs=2, space="PSUM"))

            idnbf = consts.tile([128, 128], dt.bfloat16, tag="idnbf")
            make_identity(nc, idnbf[:])

            wt = {}
            bias = {}
            for p in ("theta", "phi", "g"):
                wt[p] = consts.tile([96, 2 * O], dt.float16, tag=f"wt_{p}",
                                    name=f"wt_{p}")
                nc.sync.dma_start(wt[p][:, 0:O], wt_d[p][0:96, :])
                nc.sync.dma_start(wt[p][:, O:2 * O], wt_d[p][96:192, :])
                bias[p] = consts.tile([O, 1], dt.float32, tag=f"b_{p}",
                                      name=f"b_{p}")
                nc.sync.dma_start(bias[p][:], b_d[p][:])
            wWT = consts.tile([O, C], dt.bfloat16, tag="wWT")
            nc.sync.dma_start(wWT[:], wWT_d[:])
            bW = [consts.tile([96, 1], dt.float32, tag=f"bW{h}", name=f"bW{h}")
                  for h in (0, 1)]
            for h in (0, 1):
                nc.sync.dma_start(bW[h][:], bW_d[96 * h:96 * h + 96, :])

            cneg45 = consts.tile([128, 1], dt.float32, tag="cneg45")
            nc.vector.memset(cneg45[:], -45.0)

            theta_sb = acts.tile([O, N], dt.float16, tag="theta")
            phi_sb = acts.tile([O, N], dt.float16, tag="phi")
            gt_ones = acts.tile([128, 97 * NMC], dt.bfloat16, tag="gt")
            nc.vector.memset(gt_ones[:], 1.0)
            expS = expp.tile([128, NMC * QS], dt.bfloat16, tag="expS")

            # x halves: fp16 for matmul (first: it gates everything),
            # fp32 for the residual
            xh = [xpool.tile([96, N], dt.float16, tag=f"xh{h}", name=f"xh{h}")
                  for h in (0, 1)]
            xf = [xpool.tile([96, N], dt.float32, tag=f"xf{h}", name=f"xf{h}")
                  for h in (0, 1)]
            for j in range(NQ):
                cs = slice(j * QS, (j + 1) * QS)
                for h in (0, 1):
                    nc.sync.dma_start(xh[h][:, cs], xh_d[96 * h:96 * h + 96, cs])
            for j in range(NQ):
                cs = slice(j * QS, (j + 1) * QS)
                for h in (0, 1):
                    nc.sync.dma_start(xf[h][:, cs], x_d[96 * h:96 * h + 96, cs])
            for h in (0, 1):
                nc.vector.tensor_scalar_add(xf[h][:], xf[h][:], bW[h][:])

            # ---------------- P1: projections (through shared pools) -----
            g_sb = acts.tile([O, N], dt.bfloat16, tag="gsb")
            for pr in range(NQ // 2):
                for p, dst in (("phi", phi_sb), ("g", g_sb),
                               ("theta", theta_sb)):
                    pcs = slice(pr * 2 * QS, (pr + 1) * 2 * QS)
                    ps = ps_qk.tile([128, 2 * QS], dt.float32, tag="qk",
                                    name=f"proj_{p}_{pr}")
                    for k in (0, 1):
                        cs = slice((2 * pr + k) * QS, (2 * pr + k + 1) * QS)
                        nc.tensor.matmul(ps[0:O, k * QS:(k + 1) * QS],
                                         wt[p][:, 0:O], xh[0][:, cs],
                                         start=True, stop=False)
                        nc.tensor.matmul(ps[0:O, k * QS:(k + 1) * QS],
                                         wt[p][:, O:2 * O], xh[1][:, cs],
                                         start=False, stop=True)
                    nc.vector.tensor_scalar_add(dst[:, pcs], ps[0:O, :],
                                                bias[p][:])
                    if p == "g":
                        # transpose the 8 m-chunks of this g span into
                        # gt_ones (col 96 of each 97-chunk stays 1.0)
                        for t in (0, 1):
                            trt = ps_qk.tile([128, 2 * QS], dt.float32,
                                             tag="qk", name=f"gtr_{pr}_{t}")
                            trb = trt[:, 0:192].bitcast(dt.bfloat16)
                            for u in range(4):
                                mc = 8 * pr + 4 * t + u
                                nc.tensor.transpose(
                                    trb[:, 96 * u:96 * u + 96],
                                    g_sb[:, 128 * mc:128 * mc + 128],
                                    idnbf[0:96, 0:96])
                            a = 8 * pr + 4 * t
                            dstv = gt_ones[:, 97 * a:97 * (a + 4)].rearrange(
                                "p (c w) -> p c w", w=97)[:, :, 0:96]
                            srcv = trb[:, 0:384].rearrange(
                                "p (c w) -> p c w", w=96)
                            nc.vector.tensor_copy(dstv, srcv)

            # ---------------- P2: attention (tails pipelined 1 behind) ---
            def tail(qs, ypsum):
                qcols = slice(qs * QS, (qs + 1) * QS)
                y_bf = ypool.tile([O, QS], dt.bfloat16, tag="ybf",
                                  name=f"ybf_{qs}")
                nc.vector.tensor_copy(y_bf[:], ypsum[0:96, :])
                linv_sb = ypool.tile([1, QS], dt.float32, tag="linv",
                                     name=f"linv_{qs}")
                nc.vector.reciprocal(linv_sb[:], ypsum[96:97, :])
                li_sb = ypool.tile([O, QS], dt.float32, tag="lisb",
                                   name=f"lisb_{qs}")
                nc.gpsimd.partition_broadcast(li_sb[:], linv_sb[:])
                for h in (0, 1):
                    pso = ps_out.tile([96, QS], dt.float32, tag="pout",
                                      name=f"pout_{qs}_{h}")
                    nc.tensor.matmul(pso[:], wWT[:, 96 * h:96 * h + 96],
                                     y_bf[:], start=True, stop=True)
                    ob = outp.tile([96, QS], dt.float32, tag="ob",
                                   name=f"ob_{qs}_{h}")
                    nc.vector.tensor_mul(ob[:], pso[:], li_sb[:])
                    nc.vector.tensor_add(ob[:], ob[:], xf[h][:, qcols])
                    nc.sync.dma_start(out_d[96 * h:96 * h + 96, qcols], ob[:])

            prev = None
            for qs in range(NQ):
                qcols = slice(qs * QS, (qs + 1) * QS)
                ypsum = ps_pv.tile([97, QS], dt.float32, tag="pv",
                                   name=f"pv_{qs}")
                for pr in range(NMC // 2):
                    ps = ps_qk.tile([128, 2 * QS], dt.float32, tag="qk",
                                    name=f"qk_{qs}_{pr}")
                    for k in (0, 1):
                        mc = 2 * pr + k
                        nc.tensor.matmul(
                            ps[:, k * QS:(k + 1) * QS],
                            phi_sb[:, 128 * mc:128 * mc + 128],
                            theta_sb[:, qcols], start=True, stop=True)
                    nc.scalar.activation(
                        expS[:, 1024 * pr:1024 * pr + 1024], ps[:],
                        AF.Exp, bias=cneg45[:])
                    for k in (0, 1):
                        mc = 2 * pr + k
                        nc.tensor.matmul(
                            ypsum[:], gt_ones[:, 97 * mc:97 * mc + 97],
                            expS[:, 512 * mc:512 * mc + 512],
                            start=(mc == 0), stop=(mc == NMC - 1))
                    if prev is not None and pr == 1:
                        tail(*prev)
                        prev = None
                prev = (qs, ypsum)
            tail(*prev)

    nc.compile()
    return nc


def _get_nc():
    if "nc" not in _CACHE:
        _CACHE["nc"] = _build()
    return _CACHE["nc"]


LAST_RESULTS = None


def kernel(x, g_w, g_b, theta_w, theta_b, phi_w, phi_b, W_w, W_b):
    global LAST_RESULTS
    from concourse.bass_utils import run_bass_kernel_spmd

    nc = _get_nc()

    x = np.asarray(x, dtype=np.float32)
    common = {
        "wt_theta": np.ascontiguousarray(np.asarray(theta_w).T).astype(np.float16),
        "wt_phi": np.ascontiguousarray(np.asarray(phi_w).T).astype(np.float16),
        "wt_g": np.ascontiguousarray(np.asarray(g_w).T).astype(np.float16),
        "w_WT": np.ascontiguousarray(np.asarray(W_w).T).astype(ml_dtypes.bfloat16),
        "b_theta": np.asarray(theta_b, dtype=np.float32).reshape(O, 1),
        "b_phi": np.asarray(phi_b, dtype=np.float32).reshape(O, 1),
        "b_g": np.asarray(g_b, dtype=np.float32).reshape(O, 1),
        "b_W": np.asarray(W_b, dtype=np.float32).reshape(C, 1),
    }
    in_maps = []
    for b in range(B):
        xb = np.ascontiguousarray(x[b].reshape(C, N))
        in_maps.append({"x": xb, "xh": xb.astype(np.float16), **common})
    res = run_bass_kernel_spmd(nc, in_maps, list(range(N_CORES)))
    LAST_RESULTS = res
    out = np.stack([res.results[b]["out"].reshape(C, HH, WW) for b in range(B)])
    return out.astype(np.float32)


# revision 21
# speedup vs baseline: 1.2154x; 1.0234x over previous
"""Non-local block (self-attention over 64x64 spatial map) on 8 NeuronCores.

Sharding: data-parallel over batch (B=8 -> 1 image per core). Each core runs
the full N=4096 attention for its image; no collectives.

Per-core layout strategy:
  - theta_x/phi_x stored [O=96, N=4096] fp16; scores computed TRANSPOSED
    (S^T[m, q] chunks) so softmax denominators come out of the PE itself via
    an appended ones-column on the PV lhsT.
  - exp(S - 45) on ScalarE in [128, 1024] chunks (constant shift cancels
    exactly in softmax; bigger chunks amortize the ACT access bubble).
  - PV: lhsT = [g^T | ones] [128m, 97], rhs = expS^T [128m, 512q]
    -> y_u [97, 512] accumulated over 32 m-chunks; row 96 = softmax sums.
  - normalization commutes with the linear out-projection: 1/l is broadcast
    across partitions with a single-pass float32r PE outer product, applied
    after W^T y_u, off the critical path (tails software-pipelined one
    q-super behind the matmul stream).
"""

import numpy as np
import ml_dtypes

B, C, O = 8, 192, 96
HH, WW = 64, 64
N = HH * WW           # 4096
NQ = 8                # q-supers of 512
QS = 512
NMC = N // 128        # 32 m-chunks
N_CORES = 8

_CACHE = {}


def _build():
    from contextlib import ExitStack
    import concourse.tile as tile
    from concourse import bacc, mybir
    from concourse.masks import make_identity

    dt = mybir.dt
    AF = mybir.ActivationFunctionType

    nc = bacc.Bacc("TRN2", target_bir_lowering=False, debug=False,
                   num_devices=N_CORES)

    x_d = nc.dram_tensor("x", [C, N], dt.float32, kind="ExternalInput").ap()
    xh_d = nc.dram_tensor("xh", [C, N], dt.float16, kind="ExternalInput").ap()
    wt_d = {}
    b_d = {}
    for p in ("theta", "phi", "g"):
        wt_d[p] = nc.dram_tensor(f"wt_{p}", [C, O], dt.float16,
                                 kind="ExternalInput").ap()
        b_d[p] = nc.dram_tensor(f"b_{p}", [O, 1], dt.float32,
                                kind="ExternalInput").ap()
    wWT_d = nc.dram_tensor("w_WT", [O, C], dt.bfloat16, kind="ExternalInput").ap()
    bW_d = nc.dram_tensor("b_W", [C, 1], dt.float32, kind="ExternalInput").ap()
    out_d = nc.dram_tensor("out", [C, N], dt.float32, kind="ExternalOutput").ap()

    with tile.TileContext(nc) as tc:
        with ExitStack() as ctx:
            # ---------------- SBUF pools ----------------
            consts = ctx.enter_context(tc.tile_pool(name="consts", bufs=1))
            xpool = ctx.enter_context(tc.tile_pool(name="x", bufs=1))
            acts = ctx.enter_context(tc.tile_pool(name="acts", bufs=1))
            expp = ctx.enter_context(tc.tile_pool(name="exp", bufs=1))
            ypool = ctx.enter_context(tc.tile_pool(name="y", bufs=3))
            outp = ctx.enter_context(tc.tile_pool(name="outsb", bufs=3))
            # ---------------- PSUM pools (shared by all phases) ----------
            ps_qk = ctx.enter_context(
                tc.tile_pool(name="ps_qk", bufs=2, space="PSUM"))
            ps_pv = ctx.enter_context(
                tc.tile_pool(name="ps_pv", bufs=2, space="PSUM"))
            ps_out = ctx.enter_context(
                tc.tile_pool(name="ps_out", bufs=2, space="PSUM"))

            idnbf = consts.tile([128, 128], dt.bfloat16, tag="idnbf")
            make_identity(nc, idnbf[:])

            wt = {}
            bias = {}
            for p in ("theta", "phi", "g"):
                wt[p] = consts.tile([96, 2 * O], dt.float16, tag=f"wt_{p}",
                                    name=f"wt_{p}")
                nc.gpsimd.dma_start(wt[p][:, 0:O], wt_d[p][0:96, :])
                nc.gpsimd.dma_start(wt[p][:, O:2 * O], wt_d[p][96:192, :])
                bias[p] = consts.tile([O, 1], dt.float32, tag=f"b_{p}",
                                      name=f"b_{p}")
                nc.gpsimd.dma_start(bias[p][:], b_d[p][:])
            wWT = consts.tile([O, C], dt.bfloat16, tag="wWT")
            nc.gpsimd.dma_start(wWT[:], wWT_d[:])
            bW = [consts.tile([96, 1], dt.float32, tag=f"bW{h}", name=f"bW{h}")
                  for h in (0, 1)]
            for h in (0, 1):
                nc.gpsimd.dma_start(bW[h][:], bW_d[96 * h:96 * h + 96, :])

            cneg45 = consts.tile([128, 1], dt.float32, tag="cneg45")
            nc.vector.memset(cneg45[:], -45.0)

            theta_sb = acts.tile([O, N], dt.float16, tag="theta")
            phi_sb = acts.tile([O, N], dt.float16, tag="phi")
            gt_ones = acts.tile([128, 97 * NMC], dt.bfloat16, tag="gt")
            nc.vector.memset(gt_ones[:], 1.0)
            expS = expp.tile([128, NMC * QS], dt.bfloat16, tag="expS")

            # x halves: fp16 for matmul (first: it gates everything),
            # fp32 for the residual
            xh = [xpool.tile([96, N], dt.float16, tag=f"xh{h}", name=f"xh{h}")
                  for h in (0, 1)]
            xf = [xpool.tile([96, N], dt.float32, tag=f"xf{h}", name=f"xf{h}")
                  for h in (0, 1)]
            for j in range(NQ):
                cs = slice(j * QS, (j + 1) * QS)
                for h in (0, 1):
                    nc.sync.dma_start(xh[h][:, cs], xh_d[96 * h:96 * h + 96, cs])
            for j in range(NQ):
                cs = slice(j * QS, (j + 1) * QS)
                for h in (0, 1):
                    nc.gpsimd.dma_start(xf[h][:, cs], x_d[96 * h:96 * h + 96, cs])
            for h in (0, 1):
                nc.vector.tensor_scalar_add(xf[h][:], xf[h][:], bW[h][:])

            # ---------------- P1 + P2 ----------------
            g_sb = acts.tile([O, N], dt.bfloat16, tag="gsb")

            def emit_proj(pr):
                for p, dst in (("phi", phi_sb), ("g", g_sb),
                               ("theta", theta_sb)):
                    pcs = slice(pr * 2 * QS, (pr + 1) * 2 * QS)
                    ps = ps_qk.tile([128, 2 * QS], dt.float32, tag="qk",
                                    name=f"proj_{p}_{pr}")
                    for k in (0, 1):
                        cs = slice((2 * pr + k) * QS, (2 * pr + k + 1) * QS)
                        nc.tensor.matmul(ps[0:O, k * QS:(k + 1) * QS],
                                         wt[p][:, 0:O], xh[0][:, cs],
                                         start=True, stop=False)
                        nc.tensor.matmul(ps[0:O, k * QS:(k + 1) * QS],
                                         wt[p][:, O:2 * O], xh[1][:, cs],
                                         start=False, stop=True)
                    nc.vector.tensor_scalar_add(dst[:, pcs], ps[0:O, :],
                                                bias[p][:])
                    if p == "g":
                        # transpose this span's 8 m-chunks into gt_ones
                        # (col 96 of each 97-chunk stays 1.0)
                        for t in (0, 1):
                            trt = ps_qk.tile([128, 2 * QS], dt.float32,
                                             tag="qk", name=f"gtr_{pr}_{t}")
                            trb = trt[:, 0:192].bitcast(dt.bfloat16)
                            for u in range(4):
                                mc = 8 * pr + 4 * t + u
                                nc.tensor.transpose(
                                    trb[:, 96 * u:96 * u + 96],
                                    g_sb[:, 128 * mc:128 * mc + 128],
                                    idnbf[0:96, 0:96])
                            a = 8 * pr + 4 * t
                            dstv = gt_ones[:, 97 * a:97 * (a + 4)].rearrange(
                                "p (c w) -> p c w", w=97)[:, :, 0:96]
                            srcv = trb[:, 0:384].rearrange(
                                "p (c w) -> p c w", w=96)
                            nc.vector.tensor_copy(dstv, srcv)

            def emit_pair(qs, pr, ypsum):
                ps = ps_qk.tile([128, 2 * QS], dt.float32, tag="qk",
                                name=f"qk_{qs}_{pr}")
                qcols = slice(qs * QS, (qs + 1) * QS)
                for k in (0, 1):
                    mc = 2 * pr + k
                    nc.tensor.matmul(
                        ps[:, k * QS:(k + 1) * QS],
                        phi_sb[:, 128 * mc:128 * mc + 128],
                        theta_sb[:, qcols], start=True, stop=True)
                nc.scalar.activation(
                    expS[:, 1024 * pr:1024 * pr + 1024], ps[:],
                    AF.Exp, bias=cneg45[:])
                for k in (0, 1):
                    mc = 2 * pr + k
                    nc.tensor.matmul(
                        ypsum[:], gt_ones[:, 97 * mc:97 * mc + 97],
                        expS[:, 512 * mc:512 * mc + 512],
                        start=(mc == 0), stop=(mc == NMC - 1))

            def tail_a(qs, ypsum):
                y_bf = ypool.tile([O, QS], dt.bfloat16, tag="ybf",
                                  name=f"ybf_{qs}")
                nc.vector.tensor_copy(y_bf[:], ypsum[0:96, :])
                linv_sb = ypool.tile([1, QS], dt.float32, tag="linv",
                                     name=f"linv_{qs}")
                nc.vector.reciprocal(linv_sb[:], ypsum[96:97, :])
                li_sb = ypool.tile([O, QS], dt.float32, tag="lisb",
                                   name=f"lisb_{qs}")
                nc.gpsimd.partition_broadcast(li_sb[:], linv_sb[:])
                return y_bf, li_sb

            def tail_h(qs, h, y_bf, li_sb):
                qcols = slice(qs * QS, (qs + 1) * QS)
                pso = ps_out.tile([96, QS], dt.float32, tag="pout",
                                  name=f"pout_{qs}_{h}")
                nc.tensor.matmul(pso[:], wWT[:, 96 * h:96 * h + 96],
                                 y_bf[:], start=True, stop=True)
                ob = outp.tile([96, QS], dt.float32, tag="ob",
                               name=f"ob_{qs}_{h}")
                nc.vector.tensor_mul(ob[:], pso[:], li_sb[:])
                nc.vector.tensor_add(ob[:], ob[:], xf[h][:, qcols])
                nc.sync.dma_start(out_d[96 * h:96 * h + 96, qcols], ob[:])

            ypsums = {}
            prev = None        # (qs, ypsum) awaiting tail
            tstate = None      # (qs, y_bf, li_sb) mid-tail

            def tail_step(pr):
                nonlocal prev, tstate
                if pr == 1 and prev is not None:
                    pq, pyp = prev
                    y_bf, li_sb = tail_a(pq, pyp)
                    tstate = (pq, y_bf, li_sb)
                    prev = None
                elif pr == 2 and tstate is not None:
                    tail_h(tstate[0], 0, tstate[1], tstate[2])
                elif pr == 3 and tstate is not None:
                    tail_h(tstate[0], 1, tstate[1], tstate[2])
                    tstate = None

            for qs in range(NQ):
                ypsum = ps_pv.tile([97, QS], dt.float32, tag="pv",
                                   name=f"pv_{qs}")
                for pr in range(NMC // 2):
                    if qs == 0 and pr % 4 == 0:
                        emit_proj(pr // 4)
                    emit_pair(qs, pr, ypsum)
                    if qs > 0:
                        tail_step(pr)
                prev = (qs, ypsum)
            y_bf, li_sb = tail_a(*prev)
            tail_h(NQ - 1, 0, y_bf, li_sb)
            tail_h(NQ - 1, 1, y_bf, li_sb)

    nc.compile()
    return nc


def _get_nc():
    if "nc" not in _CACHE:
        _CACHE["nc"] = _build()
    return _CACHE["nc"]


LAST_RESULTS = None


def kernel(x, g_w, g_b, theta_w, theta_b, phi_w, phi_b, W_w, W_b):
    global LAST_RESULTS
    from concourse.bass_utils import run_bass_kernel_spmd

    nc = _get_nc()

    x = np.asarray(x, dtype=np.float32)
    common = {
        "wt_theta": np.ascontiguousarray(np.asarray(theta_w).T).astype(np.float16),
        "wt_phi": np.ascontiguousarray(np.asarray(phi_w).T).astype(np.float16),
        "wt_g": np.ascontiguousarray(np.asarray(g_w).T).astype(np.float16),
        "w_WT": np.ascontiguousarray(np.asarray(W_w).T).astype(ml_dtypes.bfloat16),
        "b_theta": np.asarray(theta_b, dtype=np.float32).reshape(O, 1),
        "b_phi": np.asarray(phi_b, dtype=np.float32).reshape(O, 1),
        "b_g": np.asarray(g_b, dtype=np.float32).reshape(O, 1),
        "b_W": np.asarray(W_b, dtype=np.float32).reshape(C, 1),
    }
    in_maps = []
    for b in range(B):
        xb = np.ascontiguousarray(x[b].reshape(C, N))
        in_maps.append({"x": xb, "xh": xb.astype(np.float16), **common})
    res = run_bass_kernel_spmd(nc, in_maps, list(range(N_CORES)))
    LAST_RESULTS = res
    out = np.stack([res.results[b]["out"].reshape(C, HH, WW) for b in range(B)])
    return out.astype(np.float32)


# revision 22
# speedup vs baseline: 1.2181x; 1.0022x over previous
"""Non-local block (self-attention over 64x64 spatial map) on 8 NeuronCores.

Sharding: data-parallel over batch (B=8 -> 1 image per core). Each core runs
the full N=4096 attention for its image; no collectives.

Per-core layout strategy:
  - theta_x/phi_x stored [O=96, N=4096] fp16; scores computed TRANSPOSED
    (S^T[m, q] chunks) so softmax denominators come out of the PE itself via
    an appended ones-column on the PV lhsT.
  - exp(S - 45) on ScalarE in [128, 1024] chunks (constant shift cancels
    exactly in softmax; bigger chunks amortize the ACT access bubble).
  - PV: lhsT = [g^T | ones] [128m, 97], rhs = expS^T [128m, 512q]
    -> y_u [97, 512] accumulated over 32 m-chunks; row 96 = softmax sums.
  - normalization commutes with the linear out-projection: 1/l is broadcast
    across partitions with a single-pass float32r PE outer product, applied
    after W^T y_u, off the critical path (tails software-pipelined one
    q-super behind the matmul stream).
"""

import numpy as np
import ml_dtypes

B, C, O = 8, 192, 96
HH, WW = 64, 64
N = HH * WW           # 4096
NQ = 8                # q-supers of 512
QS = 512
NMC = N // 128        # 32 m-chunks
N_CORES = 8

_CACHE = {}


def _build():
    from contextlib import ExitStack
    import concourse.tile as tile
    from concourse import bacc, mybir
    from concourse.masks import make_identity

    dt = mybir.dt
    AF = mybir.ActivationFunctionType

    nc = bacc.Bacc("TRN2", target_bir_lowering=False, debug=False,
                   num_devices=N_CORES)

    x_d = nc.dram_tensor("x", [C, N], dt.float32, kind="ExternalInput").ap()
    xh_d = nc.dram_tensor("xh", [C, N], dt.float16, kind="ExternalInput").ap()
    wt_d = {}
    b_d = {}
    for p in ("theta", "phi", "g"):
        wt_d[p] = nc.dram_tensor(f"wt_{p}", [C, O], dt.float16,
                                 kind="ExternalInput").ap()
        b_d[p] = nc.dram_tensor(f"b_{p}", [O, 1], dt.float32,
                                kind="ExternalInput").ap()
    wWT_d = nc.dram_tensor("w_WT", [O, C], dt.bfloat16, kind="ExternalInput").ap()
    bW_d = nc.dram_tensor("b_W", [C, 1], dt.float32, kind="ExternalInput").ap()
    out_d = nc.dram_tensor("out", [C, N], dt.float32, kind="ExternalOutput").ap()

    with tile.TileContext(nc) as tc:
        with ExitStack() as ctx:
            # ---------------- SBUF pools ----------------
            consts = ctx.enter_context(tc.tile_pool(name="consts", bufs=1))
            xpool = ctx.enter_context(tc.tile_pool(name="x", bufs=1))
            acts = ctx.enter_context(tc.tile_pool(name="acts", bufs=1))
            expp = ctx.enter_context(tc.tile_pool(name="exp", bufs=1))
            ypool = ctx.enter_context(tc.tile_pool(name="y", bufs=3))
            outp = ctx.enter_context(tc.tile_pool(name="outsb", bufs=3))
            # ---------------- PSUM pools (shared by all phases) ----------
            ps_qk = ctx.enter_context(
                tc.tile_pool(name="ps_qk", bufs=2, space="PSUM"))
            ps_pv = ctx.enter_context(
                tc.tile_pool(name="ps_pv", bufs=2, space="PSUM"))
            ps_out = ctx.enter_context(
                tc.tile_pool(name="ps_out", bufs=2, space="PSUM"))

            idnbf = consts.tile([128, 128], dt.bfloat16, tag="idnbf")
            make_identity(nc, idnbf[:])

            wt = {}
            bias = {}
            for p in ("theta", "phi", "g"):
                wt[p] = consts.tile([96, 2 * O], dt.float16, tag=f"wt_{p}",
                                    name=f"wt_{p}")
                nc.gpsimd.dma_start(wt[p][:, 0:O], wt_d[p][0:96, :])
                nc.gpsimd.dma_start(wt[p][:, O:2 * O], wt_d[p][96:192, :])
                bias[p] = consts.tile([O, 1], dt.float32, tag=f"b_{p}",
                                      name=f"b_{p}")
                nc.gpsimd.dma_start(bias[p][:], b_d[p][:])
            wWT = consts.tile([O, C], dt.bfloat16, tag="wWT")
            nc.gpsimd.dma_start(wWT[:], wWT_d[:])
            bW = [consts.tile([96, 1], dt.float32, tag=f"bW{h}", name=f"bW{h}")
                  for h in (0, 1)]
            for h in (0, 1):
                nc.gpsimd.dma_start(bW[h][:], bW_d[96 * h:96 * h + 96, :])

            cneg45 = consts.tile([128, 1], dt.float32, tag="cneg45")
            nc.vector.memset(cneg45[:], -45.0)

            theta_sb = acts.tile([O, N], dt.float16, tag="theta")
            phi_sb = acts.tile([O, N], dt.float16, tag="phi")
            gt_ones = acts.tile([128, 97 * NMC], dt.bfloat16, tag="gt")
            nc.vector.memset(gt_ones[:], 1.0)
            expS = expp.tile([128, NMC * QS], dt.bfloat16, tag="expS")

            # x halves: fp16 for matmul (first: it gates everything),
            # fp32 for the residual
            xh = [xpool.tile([96, N], dt.float16, tag=f"xh{h}", name=f"xh{h}")
                  for h in (0, 1)]
            xf = [xpool.tile([96, N], dt.float32, tag=f"xf{h}", name=f"xf{h}")
                  for h in (0, 1)]
            for j in range(NQ):
                cs = slice(j * QS, (j + 1) * QS)
                for h in (0, 1):
                    nc.sync.dma_start(xh[h][:, cs], xh_d[96 * h:96 * h + 96, cs])
            for j in range(NQ):
                cs = slice(j * QS, (j + 1) * QS)
                for h in (0, 1):
                    nc.gpsimd.dma_start(xf[h][:, cs], x_d[96 * h:96 * h + 96, cs])
            for h in (0, 1):
                nc.vector.tensor_scalar_add(xf[h][:], xf[h][:], bW[h][:])

            # ---------------- P1 + P2 ----------------
            g_sb = acts.tile([O, N], dt.bfloat16, tag="gsb")

            def emit_proj(p, pr):
                for dst in ({"phi": phi_sb, "g": g_sb,
                             "theta": theta_sb}[p],):
                    pcs = slice(pr * 2 * QS, (pr + 1) * 2 * QS)
                    ps = ps_qk.tile([128, 2 * QS], dt.float32, tag="qk",
                                    name=f"proj_{p}_{pr}")
                    for k in (0, 1):
                        cs = slice((2 * pr + k) * QS, (2 * pr + k + 1) * QS)
                        nc.tensor.matmul(ps[0:O, k * QS:(k + 1) * QS],
                                         wt[p][:, 0:O], xh[0][:, cs],
                                         start=True, stop=False)
                        nc.tensor.matmul(ps[0:O, k * QS:(k + 1) * QS],
                                         wt[p][:, O:2 * O], xh[1][:, cs],
                                         start=False, stop=True)
                    nc.vector.tensor_scalar_add(dst[:, pcs], ps[0:O, :],
                                                bias[p][:])
                    if p == "g":
                        # transpose this span's 8 m-chunks into gt_ones
                        # (col 96 of each 97-chunk stays 1.0)
                        for t in (0, 1):
                            trt = ps_qk.tile([128, 2 * QS], dt.float32,
                                             tag="qk", name=f"gtr_{pr}_{t}")
                            trb = trt[:, 0:192].bitcast(dt.bfloat16)
                            for u in range(4):
                                mc = 8 * pr + 4 * t + u
                                nc.tensor.transpose(
                                    trb[:, 96 * u:96 * u + 96],
                                    g_sb[:, 128 * mc:128 * mc + 128],
                                    idnbf[0:96, 0:96])
                            a = 8 * pr + 4 * t
                            dstv = gt_ones[:, 97 * a:97 * (a + 4)].rearrange(
                                "p (c w) -> p c w", w=97)[:, :, 0:96]
                            srcv = trb[:, 0:384].rearrange(
                                "p (c w) -> p c w", w=96)
                            nc.vector.tensor_copy(dstv, srcv)

            def emit_pair(qs, pr, ypsum):
                ps = ps_qk.tile([128, 2 * QS], dt.float32, tag="qk",
                                name=f"qk_{qs}_{pr}")
                qcols = slice(qs * QS, (qs + 1) * QS)
                for k in (0, 1):
                    mc = 2 * pr + k
                    nc.tensor.matmul(
                        ps[:, k * QS:(k + 1) * QS],
                        phi_sb[:, 128 * mc:128 * mc + 128],
                        theta_sb[:, qcols], start=True, stop=True)
                nc.scalar.activation(
                    expS[:, 1024 * pr:1024 * pr + 1024], ps[:],
                    AF.Exp, bias=cneg45[:])
                for k in (0, 1):
                    mc = 2 * pr + k
                    nc.tensor.matmul(
                        ypsum[:], gt_ones[:, 97 * mc:97 * mc + 97],
                        expS[:, 512 * mc:512 * mc + 512],
                        start=(mc == 0), stop=(mc == NMC - 1))

            def tail_a(qs, ypsum):
                y_bf = ypool.tile([O, QS], dt.bfloat16, tag="ybf",
                                  name=f"ybf_{qs}")
                nc.vector.tensor_copy(y_bf[:], ypsum[0:96, :])
                linv_sb = ypool.tile([1, QS], dt.float32, tag="linv",
                                     name=f"linv_{qs}")
                nc.vector.reciprocal(linv_sb[:], ypsum[96:97, :])
                li_sb = ypool.tile([O, QS], dt.float32, tag="lisb",
                                   name=f"lisb_{qs}")
                nc.gpsimd.partition_broadcast(li_sb[:], linv_sb[:])
                return y_bf, li_sb

            def tail_h(qs, h, y_bf, li_sb):
                qcols = slice(qs * QS, (qs + 1) * QS)
                pso = ps_out.tile([96, QS], dt.float32, tag="pout",
                                  name=f"pout_{qs}_{h}")
                nc.tensor.matmul(pso[:], wWT[:, 96 * h:96 * h + 96],
                                 y_bf[:], start=True, stop=True)
                ob = outp.tile([96, QS], dt.float32, tag="ob",
                               name=f"ob_{qs}_{h}")
                nc.vector.tensor_mul(ob[:], pso[:], li_sb[:])
                nc.vector.tensor_add(ob[:], ob[:], xf[h][:, qcols])
                nc.sync.dma_start(out_d[96 * h:96 * h + 96, qcols], ob[:])

            ypsums = {}
            prev = None        # (qs, ypsum) awaiting tail
            tstate = None      # (qs, y_bf, li_sb) mid-tail

            def tail_step(pr):
                nonlocal prev, tstate
                if pr == 1 and prev is not None:
                    pq, pyp = prev
                    y_bf, li_sb = tail_a(pq, pyp)
                    tstate = (pq, y_bf, li_sb)
                    prev = None
                elif pr == 2 and tstate is not None:
                    tail_h(tstate[0], 0, tstate[1], tstate[2])
                elif pr == 3 and tstate is not None:
                    tail_h(tstate[0], 1, tstate[1], tstate[2])
                    tstate = None

            for pr in range(4):
                emit_proj("phi", pr)
            for pr in range(4):
                emit_proj("g", pr)
            emit_proj("theta", 0)
            for qs in range(NQ):
                ypsum = ps_pv.tile([97, QS], dt.float32, tag="pv",
                                   name=f"pv_{qs}")
                for pr in range(NMC // 2):
                    if qs == 0 and pr in (4, 8, 12):
                        emit_proj("theta", pr // 4)
                    emit_pair(qs, pr, ypsum)
                    if qs > 0:
                        tail_step(pr)
                prev = (qs, ypsum)
            y_bf, li_sb = tail_a(*prev)
            tail_h(NQ - 1, 0, y_bf, li_sb)
            tail_h(NQ - 1, 1, y_bf, li_sb)

    nc.compile()
    return nc


def _get_nc():
    if "nc" not in _CACHE:
        _CACHE["nc"] = _build()
    return _CACHE["nc"]


LAST_RESULTS = None


def kernel(x, g_w, g_b, theta_w, theta_b, phi_w, phi_b, W_w, W_b):
    global LAST_RESULTS
    from concourse.bass_utils import run_bass_kernel_spmd

    nc = _get_nc()

    x = np.asarray(x, dtype=np.float32)
    common = {
        "wt_theta": np.ascontiguousarray(np.asarray(theta_w).T).astype(np.float16),
        "wt_phi": np.ascontiguousarray(np.asarray(phi_w).T).astype(np.float16),
        "wt_g": np.ascontiguousarray(np.asarray(g_w).T).astype(np.float16),
        "w_WT": np.ascontiguousarray(np.asarray(W_w).T).astype(ml_dtypes.bfloat16),
        "b_theta": np.asarray(theta_b, dtype=np.float32).reshape(O, 1),
        "b_phi": np.asarray(phi_b, dtype=np.float32).reshape(O, 1),
        "b_g": np.asarray(g_b, dtype=np.float32).reshape(O, 1),
        "b_W": np.asarray(W_b, dtype=np.float32).reshape(C, 1),
    }
    in_maps = []
    for b in range(B):
        xb = np.ascontiguousarray(x[b].reshape(C, N))
        in_maps.append({"x": xb, "xh": xb.astype(np.float16), **common})
    res = run_bass_kernel_spmd(nc, in_maps, list(range(N_CORES)))
    LAST_RESULTS = res
    out = np.stack([res.results[b]["out"].reshape(C, HH, WW) for b in range(B)])
    return out.astype(np.float32)


# revision 23
# speedup vs baseline: 1.3740x; 1.1280x over previous
"""Non-local block (self-attention over 64x64 spatial map) on 8 NeuronCores.

Sharding: data-parallel over batch (B=8 -> 1 image per core). Each core runs
the full N=4096 attention for its image; no collectives.

Per-core layout strategy:
  - theta_x/phi_x stored [O=96, N=4096] fp16; scores computed TRANSPOSED
    (S^T[m, q] chunks) so softmax denominators come out of the PE itself via
    an appended ones-column on the PV lhsT.
  - exp(S - 45) on ScalarE in [128, 1024] chunks (constant shift cancels
    exactly in softmax; bigger chunks amortize the ACT access bubble).
  - PV: lhsT = [g^T | ones] [128m, 97], rhs = expS^T [128m, 512q]
    -> y_u [97, 512] accumulated over 32 m-chunks; row 96 = softmax sums.
  - normalization commutes with the linear out-projection: 1/l is broadcast
    across partitions with a single-pass float32r PE outer product, applied
    after W^T y_u, off the critical path (tails software-pipelined one
    q-super behind the matmul stream).
"""

import numpy as np
import ml_dtypes

B, C, O = 8, 192, 96
HH, WW = 64, 64
N = HH * WW           # 4096
NQ = 8                # q-supers of 512
QS = 512
NMC = N // 128        # 32 m-chunks
N_CORES = 8

_CACHE = {}


def _build():
    from contextlib import ExitStack
    import concourse.tile as tile
    from concourse import bacc, mybir
    from concourse.masks import make_identity

    dt = mybir.dt
    AF = mybir.ActivationFunctionType

    nc = bacc.Bacc("TRN2", target_bir_lowering=False, debug=False,
                   num_devices=N_CORES)

    x_d = nc.dram_tensor("x", [C, N], dt.float32, kind="ExternalInput").ap()
    xh_d = nc.dram_tensor("xh", [C, N], dt.float16, kind="ExternalInput").ap()
    wt_d = {}
    b_d = {}
    for p in ("theta", "phi", "g"):
        wt_d[p] = nc.dram_tensor(f"wt_{p}", [C, O], dt.float16,
                                 kind="ExternalInput").ap()
        b_d[p] = nc.dram_tensor(f"b_{p}", [O, 1], dt.float32,
                                kind="ExternalInput").ap()
    wWT_d = nc.dram_tensor("w_WT", [O, C], dt.bfloat16, kind="ExternalInput").ap()
    bW_d = nc.dram_tensor("b_W", [C, 1], dt.float32, kind="ExternalInput").ap()
    out_d = nc.dram_tensor("out", [C, N], dt.float32, kind="ExternalOutput").ap()

    with tile.TileContext(nc) as tc:
        with ExitStack() as ctx:
            # ---------------- SBUF pools ----------------
            consts = ctx.enter_context(tc.tile_pool(name="consts", bufs=1))
            xpool = ctx.enter_context(tc.tile_pool(name="x", bufs=1))
            acts = ctx.enter_context(tc.tile_pool(name="acts", bufs=1))
            expp = ctx.enter_context(tc.tile_pool(name="exp", bufs=1))
            ypool = ctx.enter_context(tc.tile_pool(name="y", bufs=3))
            outp = ctx.enter_context(tc.tile_pool(name="outsb", bufs=3))
            # ---------------- PSUM pools (shared by all phases) ----------
            ps_qk = ctx.enter_context(
                tc.tile_pool(name="ps_qk", bufs=2, space="PSUM"))
            ps_pv = ctx.enter_context(
                tc.tile_pool(name="ps_pv", bufs=2, space="PSUM"))
            ps_out = ctx.enter_context(
                tc.tile_pool(name="ps_out", bufs=2, space="PSUM"))

            idnbf = consts.tile([128, 128], dt.bfloat16, tag="idnbf")
            make_identity(nc, idnbf[:])

            wt = {}
            bias = {}
            for p in ("theta", "phi", "g"):
                wt[p] = consts.tile([96, 2 * O], dt.float16, tag=f"wt_{p}",
                                    name=f"wt_{p}")
                nc.gpsimd.dma_start(wt[p][:, 0:O], wt_d[p][0:96, :])
                nc.gpsimd.dma_start(wt[p][:, O:2 * O], wt_d[p][96:192, :])
                bias[p] = consts.tile([O, 1], dt.float32, tag=f"b_{p}",
                                      name=f"b_{p}")
                nc.gpsimd.dma_start(bias[p][:], b_d[p][:])
            wWT = consts.tile([O, C], dt.bfloat16, tag="wWT")
            nc.gpsimd.dma_start(wWT[:], wWT_d[:])
            bW = [consts.tile([96, 1], dt.float32, tag=f"bW{h}", name=f"bW{h}")
                  for h in (0, 1)]
            for h in (0, 1):
                nc.gpsimd.dma_start(bW[h][:], bW_d[96 * h:96 * h + 96, :])

            cneg45 = consts.tile([128, 1], dt.float32, tag="cneg45")
            nc.vector.memset(cneg45[:], -45.0)

            theta_sb = acts.tile([O, N], dt.float16, tag="theta")
            phi_sb = acts.tile([O, N], dt.float16, tag="phi")
            gt_ones = acts.tile([128, 97 * NMC], dt.bfloat16, tag="gt")
            nc.vector.memset(gt_ones[:], 1.0)
            expS = expp.tile([128, NMC * QS], dt.bfloat16, tag="expS")

            # x halves: fp16 for matmul (first: it gates everything),
            # fp32 for the residual
            xh = [xpool.tile([96, N], dt.float16, tag=f"xh{h}", name=f"xh{h}")
                  for h in (0, 1)]
            xf = [xpool.tile([96, N], dt.float32, tag=f"xf{h}", name=f"xf{h}")
                  for h in (0, 1)]
            for j in range(NQ):
                cs = slice(j * QS, (j + 1) * QS)
                for h in (0, 1):
                    nc.sync.dma_start(xh[h][:, cs], xh_d[96 * h:96 * h + 96, cs])
            for j in range(NQ):
                cs = slice(j * QS, (j + 1) * QS)
                for h in (0, 1):
                    nc.gpsimd.dma_start(xf[h][:, cs], x_d[96 * h:96 * h + 96, cs])
            for h in (0, 1):
                nc.vector.tensor_scalar_add(xf[h][:], xf[h][:], bW[h][:])

            # ---------------- P1 + P2, software-pipelined ----------------
            # window w streams QK(w)+exp(w) on ACT while PE also runs
            # PV(w-1) (relaxes gt deadlines + empties window 0), tail(w-2),
            # and dribbles leftover projection work one item per pair.
            g_sb = acts.tile([O, N], dt.bfloat16, tag="gsb")

            def emit_proj_mms(p, pr, k, state):
                if "ps" not in state:
                    state["ps"] = ps_qk.tile([128, 2 * QS], dt.float32,
                                             tag="qk", name=f"proj_{p}_{pr}")
                ps = state["ps"]
                cs = slice((2 * pr + k) * QS, (2 * pr + k + 1) * QS)
                nc.tensor.matmul(ps[0:O, k * QS:(k + 1) * QS],
                                 wt[p][:, 0:O], xh[0][:, cs],
                                 start=True, stop=False)
                nc.tensor.matmul(ps[0:O, k * QS:(k + 1) * QS],
                                 wt[p][:, O:2 * O], xh[1][:, cs],
                                 start=False, stop=True)
                if k == 1:
                    dst = {"phi": phi_sb, "g": g_sb, "theta": theta_sb}[p]
                    pcs = slice(pr * 2 * QS, (pr + 1) * 2 * QS)
                    nc.vector.tensor_scalar_add(dst[:, pcs], ps[0:O, :],
                                                bias[p][:])

            def emit_gtr(pr, t):
                trt = ps_qk.tile([128, 2 * QS], dt.float32, tag="qk",
                                 name=f"gtr_{pr}_{t}")
                trb = trt[:, 0:192].bitcast(dt.bfloat16)
                for u in range(4):
                    mc = 8 * pr + 4 * t + u
                    nc.tensor.transpose(trb[:, 96 * u:96 * u + 96],
                                        g_sb[:, 128 * mc:128 * mc + 128],
                                        idnbf[0:96, 0:96])
                a = 8 * pr + 4 * t
                dstv = gt_ones[:, 97 * a:97 * (a + 4)].rearrange(
                    "p (c w) -> p c w", w=97)[:, :, 0:96]
                srcv = trb[:, 0:384].rearrange("p (c w) -> p c w", w=96)
                nc.vector.tensor_copy(dstv, srcv)

            def proj_items(p, pr):
                state = {}
                items = [lambda k=k: emit_proj_mms(p, pr, k, state)
                         for k in (0, 1)]
                if p == "g":
                    items += [lambda t=t: emit_gtr(pr, t) for t in (0, 1)]
                return items

            def emit_qk_exp(qs, pr):
                ps = ps_qk.tile([128, 2 * QS], dt.float32, tag="qk",
                                name=f"qk_{qs}_{pr}")
                qcols = slice(qs * QS, (qs + 1) * QS)
                for k in (0, 1):
                    mc = 2 * pr + k
                    nc.tensor.matmul(
                        ps[:, k * QS:(k + 1) * QS],
                        phi_sb[:, 128 * mc:128 * mc + 128],
                        theta_sb[:, qcols], start=True, stop=True)
                nc.scalar.activation(
                    expS[:, 1024 * pr:1024 * pr + 1024], ps[:],
                    AF.Exp, bias=cneg45[:])

            def emit_pv(qs, mc, ypsum):
                nc.tensor.matmul(
                    ypsum[:], gt_ones[:, 97 * mc:97 * mc + 97],
                    expS[:, 512 * mc:512 * mc + 512],
                    start=(mc == 0), stop=(mc == NMC - 1))

            tails = {}

            def tail_step(qs, pr, ypsum):
                if pr == 1:
                    y_bf = ypool.tile([O, QS], dt.bfloat16, tag="ybf",
                                      name=f"ybf_{qs}")
                    nc.vector.tensor_copy(y_bf[:], ypsum[0:96, :])
                    linv_sb = ypool.tile([1, QS], dt.float32, tag="linv",
                                         name=f"linv_{qs}")
                    nc.vector.reciprocal(linv_sb[:], ypsum[96:97, :])
                    li_sb = ypool.tile([O, QS], dt.float32, tag="lisb",
                                       name=f"lisb_{qs}")
                    nc.gpsimd.partition_broadcast(li_sb[:], linv_sb[:])
                    tails[qs] = (y_bf, li_sb)
                elif pr in (2, 3) and qs in tails:
                    h = pr - 2
                    y_bf, li_sb = tails[qs]
                    qcols = slice(qs * QS, (qs + 1) * QS)
                    pso = ps_out.tile([96, QS], dt.float32, tag="pout",
                                      name=f"pout_{qs}_{h}")
                    nc.tensor.matmul(pso[:], wWT[:, 96 * h:96 * h + 96],
                                     y_bf[:], start=True, stop=True)
                    ob = outp.tile([96, QS], dt.float32, tag="ob",
                                   name=f"ob_{qs}_{h}")
                    nc.vector.tensor_mul(ob[:], pso[:], li_sb[:])
                    nc.vector.tensor_add(ob[:], ob[:], xf[h][:, qcols])
                    nc.sync.dma_start(out_d[96 * h:96 * h + 96, qcols], ob[:])

            p0 = {}
            emit_proj_mms("phi", 0, 0, p0)
            emit_proj_mms("phi", 0, 1, p0)
            t0s = {}
            emit_proj_mms("theta", 0, 0, t0s)
            emit_proj_mms("theta", 0, 1, t0s)

            work = []
            for pr in (1, 2, 3):
                work += proj_items("phi", pr)
            for pr in (0, 1, 2, 3):
                work += proj_items("g", pr)
            for pr in (1, 2, 3):
                work += proj_items("theta", pr)

            ypsums = {}
            for w in range(NQ):
                if w >= 1:
                    ypsums[w - 1] = ps_pv.tile([97, QS], dt.float32, tag="pv",
                                               name=f"pv_{w - 1}")
                for pr in range(NMC // 2):
                    if w >= 1:
                        for k in (0, 1):
                            emit_pv(w - 1, 2 * pr + k, ypsums[w - 1])
                    emit_qk_exp(w, pr)
                    if work:
                        work.pop(0)()
                    if w >= 2:
                        tail_step(w - 2, pr, ypsums[w - 2])
            # epilogue: PV(7), tail(6), tail(7)
            ypsums[NQ - 1] = ps_pv.tile([97, QS], dt.float32, tag="pv",
                                        name=f"pv_{NQ - 1}")
            for pr in range(NMC // 2):
                for k in (0, 1):
                    emit_pv(NQ - 1, 2 * pr + k, ypsums[NQ - 1])
                tail_step(NQ - 2, pr, ypsums[NQ - 2])
            for pr in (1, 2, 3):
                tail_step(NQ - 1, pr, ypsums[NQ - 1])
            while work:
                work.pop(0)()

    nc.compile()
    return nc


def _get_nc():
    if "nc" not in _CACHE:
        _CACHE["nc"] = _build()
    return _CACHE["nc"]


LAST_RESULTS = None


def kernel(x, g_w, g_b, theta_w, theta_b, phi_w, phi_b, W_w, W_b):
    global LAST_RESULTS
    from concourse.bass_utils import run_bass_kernel_spmd

    nc = _get_nc()

    x = np.asarray(x, dtype=np.float32)
    common = {
        "wt_theta": np.ascontiguousarray(np.asarray(theta_w).T).astype(np.float16),
        "wt_phi": np.ascontiguousarray(np.asarray(phi_w).T).astype(np.float16),
        "wt_g": np.ascontiguousarray(np.asarray(g_w).T).astype(np.float16),
        "w_WT": np.ascontiguousarray(np.asarray(W_w).T).astype(ml_dtypes.bfloat16),
        "b_theta": np.asarray(theta_b, dtype=np.float32).reshape(O, 1),
        "b_phi": np.asarray(phi_b, dtype=np.float32).reshape(O, 1),
        "b_g": np.asarray(g_b, dtype=np.float32).reshape(O, 1),
        "b_W": np.asarray(W_b, dtype=np.float32).reshape(C, 1),
    }
    in_maps = []
    for b in range(B):
        xb = np.ascontiguousarray(x[b].reshape(C, N))
        in_maps.append({"x": xb, "xh": xb.astype(np.float16), **common})
    res = run_bass_kernel_spmd(nc, in_maps, list(range(N_CORES)))
    LAST_RESULTS = res
    out = np.stack([res.results[b]["out"].reshape(C, HH, WW) for b in range(B)])
    return out.astype(np.float32)


# revision 24
# speedup vs baseline: 1.4140x; 1.0291x over previous
"""Non-local block (self-attention over 64x64 spatial map) on 8 NeuronCores.

Sharding: data-parallel over batch (B=8 -> 1 image per core). Each core runs
the full N=4096 attention for its image; no collectives.

Per-core layout strategy:
  - theta_x/phi_x stored [O=96, N=4096] fp16; scores computed TRANSPOSED
    (S^T[m, q] chunks) so softmax denominators come out of the PE itself via
    an appended ones-column on the PV lhsT.
  - exp(S - 45) on ScalarE in [128, 1024] chunks (constant shift cancels
    exactly in softmax; bigger chunks amortize the ACT access bubble).
  - PV: lhsT = [g^T | ones] [128m, 97], rhs = expS^T [128m, 512q]
    -> y_u [97, 512] accumulated over 32 m-chunks; row 96 = softmax sums.
  - normalization commutes with the linear out-projection: 1/l is broadcast
    across partitions with a single-pass float32r PE outer product, applied
    after W^T y_u, off the critical path (tails software-pipelined one
    q-super behind the matmul stream).
"""

import numpy as np
import ml_dtypes

B, C, O = 8, 192, 96
HH, WW = 64, 64
N = HH * WW           # 4096
NQ = 8                # q-supers of 512
QS = 512
NMC = N // 128        # 32 m-chunks
N_CORES = 8

_CACHE = {}


def _build():
    from contextlib import ExitStack
    import concourse.tile as tile
    from concourse import bacc, mybir

    dt = mybir.dt
    AF = mybir.ActivationFunctionType

    nc = bacc.Bacc("TRN2", target_bir_lowering=False, debug=False,
                   num_devices=N_CORES)

    x_d = nc.dram_tensor("x", [C, N], dt.float32, kind="ExternalInput").ap()
    xh_d = nc.dram_tensor("xh", [C, N], dt.float16, kind="ExternalInput").ap()
    wt_d = {}
    b_d = {}
    for p in ("theta", "phi", "g"):
        wt_d[p] = nc.dram_tensor(f"wt_{p}", [C, O], dt.float16,
                                 kind="ExternalInput").ap()
        b_d[p] = nc.dram_tensor(f"b_{p}", [O, 1], dt.float32,
                                kind="ExternalInput").ap()
    wWT_d = nc.dram_tensor("w_WT", [O, C], dt.bfloat16, kind="ExternalInput").ap()
    bW_d = nc.dram_tensor("b_W", [C, 1], dt.float32, kind="ExternalInput").ap()
    out_d = nc.dram_tensor("out", [C, N], dt.float32, kind="ExternalOutput").ap()

    with tile.TileContext(nc) as tc:
        with ExitStack() as ctx:
            # ---------------- SBUF pools ----------------
            consts = ctx.enter_context(tc.tile_pool(name="consts", bufs=1))
            xpool = ctx.enter_context(tc.tile_pool(name="x", bufs=1))
            acts = ctx.enter_context(tc.tile_pool(name="acts", bufs=1))
            expp = ctx.enter_context(tc.tile_pool(name="exp", bufs=1))
            ypool = ctx.enter_context(tc.tile_pool(name="y", bufs=3))
            outp = ctx.enter_context(tc.tile_pool(name="outsb", bufs=3))
            # ---------------- PSUM pools (shared by all phases) ----------
            ps_qk = ctx.enter_context(
                tc.tile_pool(name="ps_qk", bufs=2, space="PSUM"))
            ps_pv = ctx.enter_context(
                tc.tile_pool(name="ps_pv", bufs=2, space="PSUM"))
            ps_out = ctx.enter_context(
                tc.tile_pool(name="ps_out", bufs=2, space="PSUM"))

            wt = {}
            bias = {}
            for p in ("theta", "phi", "g"):
                wt[p] = consts.tile([96, 2 * O], dt.float16, tag=f"wt_{p}",
                                    name=f"wt_{p}")
                nc.gpsimd.dma_start(wt[p][:, 0:O], wt_d[p][0:96, :])
                nc.gpsimd.dma_start(wt[p][:, O:2 * O], wt_d[p][96:192, :])
                bias[p] = consts.tile([O, 1], dt.float32, tag=f"b_{p}",
                                      name=f"b_{p}")
                nc.gpsimd.dma_start(bias[p][:], b_d[p][:])
            wWT = consts.tile([O, C], dt.bfloat16, tag="wWT")
            nc.gpsimd.dma_start(wWT[:], wWT_d[:])
            bW = [consts.tile([96, 1], dt.float32, tag=f"bW{h}", name=f"bW{h}")
                  for h in (0, 1)]
            for h in (0, 1):
                nc.gpsimd.dma_start(bW[h][:], bW_d[96 * h:96 * h + 96, :])

            cneg45 = consts.tile([128, 1], dt.float32, tag="cneg45")
            nc.vector.memset(cneg45[:], -45.0)

            theta_sb = acts.tile([O, N], dt.float16, tag="theta")
            phi_sb = acts.tile([O, N], dt.float16, tag="phi")
            gt_ones = acts.tile([128, 97 * NMC], dt.bfloat16, tag="gt")
            nc.vector.memset(gt_ones[:], 1.0)
            expS = expp.tile([128, NMC * QS], dt.bfloat16, tag="expS")

            # x halves: fp16 for matmul (first: it gates everything),
            # fp32 for the residual
            xh = [xpool.tile([96, N], dt.float16, tag=f"xh{h}", name=f"xh{h}")
                  for h in (0, 1)]
            xf = [xpool.tile([96, N], dt.float32, tag=f"xf{h}", name=f"xf{h}")
                  for h in (0, 1)]
            for j in range(NQ):
                cs = slice(j * QS, (j + 1) * QS)
                for h in (0, 1):
                    nc.sync.dma_start(xh[h][:, cs], xh_d[96 * h:96 * h + 96, cs])
            for j in range(NQ):
                cs = slice(j * QS, (j + 1) * QS)
                for h in (0, 1):
                    nc.gpsimd.dma_start(xf[h][:, cs], x_d[96 * h:96 * h + 96, cs])
            for h in (0, 1):
                nc.vector.tensor_scalar_add(xf[h][:], xf[h][:], bW[h][:])

            # ---------------- P1 + P2, software-pipelined ----------------
            # window w streams QK(w)+exp(w) on ACT while PE also runs
            # PV(w-1), tail(w-2), and dribbles leftover projection work.
            # g^T is produced directly by the PE (lhsT = x chunk), its bias
            # is folded into b_W on the host.

            def emit_proj_mms(p, pr, k, state):
                if "ps" not in state:
                    state["ps"] = ps_qk.tile([128, 2 * QS], dt.float32,
                                             tag="qk", name=f"proj_{p}_{pr}")
                ps = state["ps"]
                cs = slice((2 * pr + k) * QS, (2 * pr + k + 1) * QS)
                nc.tensor.matmul(ps[0:O, k * QS:(k + 1) * QS],
                                 wt[p][:, 0:O], xh[0][:, cs],
                                 start=True, stop=False)
                nc.tensor.matmul(ps[0:O, k * QS:(k + 1) * QS],
                                 wt[p][:, O:2 * O], xh[1][:, cs],
                                 start=False, stop=True)
                if k == 1:
                    dst = {"phi": phi_sb, "theta": theta_sb}[p]
                    pcs = slice(pr * 2 * QS, (pr + 1) * 2 * QS)
                    nc.vector.tensor_scalar_add(dst[:, pcs], ps[0:O, :],
                                                bias[p][:])

            def emit_gt(k2):
                # gt chunks 2*k2, 2*k2+1 via lhsT = x chunk (direct g^T)
                ps = ps_qk.tile([128, 2 * QS], dt.float32, tag="qk",
                                name=f"gt_{k2}")
                for j in (0, 1):
                    mc = 2 * k2 + j
                    mcs = slice(128 * mc, 128 * mc + 128)
                    nc.tensor.matmul(ps[:, 512 * j:512 * j + 96],
                                     xh[0][:, mcs], wt["g"][:, 0:O],
                                     start=True, stop=False)
                    nc.tensor.matmul(ps[:, 512 * j:512 * j + 96],
                                     xh[1][:, mcs], wt["g"][:, O:2 * O],
                                     start=False, stop=True)
                a = 2 * k2
                dstv = gt_ones[:, 97 * a:97 * (a + 2)].rearrange(
                    "p (c w) -> p c w", w=97)[:, :, 0:96]
                srcv = ps[:, 0:1024].rearrange(
                    "p (c w) -> p c w", w=512)[:, :, 0:96]
                nc.vector.tensor_copy(dstv, srcv)

            def proj_items(p, pr):
                state = {}
                return [lambda k=k: emit_proj_mms(p, pr, k, state)
                        for k in (0, 1)]

            def emit_qk_exp(qs, pr):
                ps = ps_qk.tile([128, 2 * QS], dt.float32, tag="qk",
                                name=f"qk_{qs}_{pr}")
                qcols = slice(qs * QS, (qs + 1) * QS)
                for k in (0, 1):
                    mc = 2 * pr + k
                    nc.tensor.matmul(
                        ps[:, k * QS:(k + 1) * QS],
                        phi_sb[:, 128 * mc:128 * mc + 128],
                        theta_sb[:, qcols], start=True, stop=True)
                nc.scalar.activation(
                    expS[:, 1024 * pr:1024 * pr + 1024], ps[:],
                    AF.Exp, bias=cneg45[:])

            def emit_pv(qs, mc, ypsum):
                nc.tensor.matmul(
                    ypsum[:], gt_ones[:, 97 * mc:97 * mc + 97],
                    expS[:, 512 * mc:512 * mc + 512],
                    start=(mc == 0), stop=(mc == NMC - 1))

            tails = {}

            def tail_step(qs, pr, ypsum):
                if pr == 1:
                    y_bf = ypool.tile([O, QS], dt.bfloat16, tag="ybf",
                                      name=f"ybf_{qs}")
                    nc.vector.tensor_copy(y_bf[:], ypsum[0:96, :])
                    linv_sb = ypool.tile([1, QS], dt.float32, tag="linv",
                                         name=f"linv_{qs}")
                    nc.vector.reciprocal(linv_sb[:], ypsum[96:97, :])
                    li_sb = ypool.tile([O, QS], dt.float32, tag="lisb",
                                       name=f"lisb_{qs}")
                    nc.gpsimd.partition_broadcast(li_sb[:], linv_sb[:])
                    tails[qs] = (y_bf, li_sb)
                elif pr in (2, 3) and qs in tails:
                    h = pr - 2
                    y_bf, li_sb = tails[qs]
                    qcols = slice(qs * QS, (qs + 1) * QS)
                    pso = ps_out.tile([96, QS], dt.float32, tag="pout",
                                      name=f"pout_{qs}_{h}")
                    nc.tensor.matmul(pso[:], wWT[:, 96 * h:96 * h + 96],
                                     y_bf[:], start=True, stop=True)
                    ob = outp.tile([96, QS], dt.float32, tag="ob",
                                   name=f"ob_{qs}_{h}")
                    nc.vector.tensor_mul(ob[:], pso[:], li_sb[:])
                    nc.vector.tensor_add(ob[:], ob[:], xf[h][:, qcols])
                    nc.sync.dma_start(out_d[96 * h:96 * h + 96, qcols], ob[:])

            p0 = {}
            emit_proj_mms("phi", 0, 0, p0)
            emit_proj_mms("phi", 0, 1, p0)
            t0s = {}
            emit_proj_mms("theta", 0, 0, t0s)
            emit_proj_mms("theta", 0, 1, t0s)

            work = []
            for pr in (1, 2, 3):
                work += proj_items("phi", pr)
            work += proj_items("theta", 1)
            work += [lambda k2=k2: emit_gt(k2) for k2 in range(NMC // 2)]
            work += proj_items("theta", 2)
            work += proj_items("theta", 3)

            ypsums = {}
            for w in range(NQ):
                if w >= 1:
                    ypsums[w - 1] = ps_pv.tile([97, QS], dt.float32, tag="pv",
                                               name=f"pv_{w - 1}")
                for pr in range(NMC // 2):
                    if not (w == 0 and pr == 0) and work:
                        work.pop(0)()
                    if w >= 1:
                        for k in (0, 1):
                            emit_pv(w - 1, 2 * pr + k, ypsums[w - 1])
                    emit_qk_exp(w, pr)
                    if w >= 2:
                        tail_step(w - 2, pr, ypsums[w - 2])
            # epilogue: PV(7), tail(6), tail(7)
            ypsums[NQ - 1] = ps_pv.tile([97, QS], dt.float32, tag="pv",
                                        name=f"pv_{NQ - 1}")
            for pr in range(NMC // 2):
                for k in (0, 1):
                    emit_pv(NQ - 1, 2 * pr + k, ypsums[NQ - 1])
                tail_step(NQ - 2, pr, ypsums[NQ - 2])
            for pr in (1, 2, 3):
                tail_step(NQ - 1, pr, ypsums[NQ - 1])
            while work:
                work.pop(0)()

    nc.compile()
    return nc


def _get_nc():
    if "nc" not in _CACHE:
        _CACHE["nc"] = _build()
    return _CACHE["nc"]


LAST_RESULTS = None


def kernel(x, g_w, g_b, theta_w, theta_b, phi_w, phi_b, W_w, W_b):
    global LAST_RESULTS
    from concourse.bass_utils import run_bass_kernel_spmd

    nc = _get_nc()

    x = np.asarray(x, dtype=np.float32)
    common = {
        "wt_theta": np.ascontiguousarray(np.asarray(theta_w).T).astype(np.float16),
        "wt_phi": np.ascontiguousarray(np.asarray(phi_w).T).astype(np.float16),
        "wt_g": np.ascontiguousarray(np.asarray(g_w).T).astype(np.float16),
        "w_WT": np.ascontiguousarray(np.asarray(W_w).T).astype(ml_dtypes.bfloat16),
        "b_theta": np.asarray(theta_b, dtype=np.float32).reshape(O, 1),
        "b_phi": np.asarray(phi_b, dtype=np.float32).reshape(O, 1),
        "b_g": np.asarray(g_b, dtype=np.float32).reshape(O, 1),
        "b_W": (np.asarray(W_b, dtype=np.float32)
                + np.asarray(W_w, dtype=np.float32)
                @ np.asarray(g_b, dtype=np.float32)).reshape(C, 1),
    }
    in_maps = []
    for b in range(B):
        xb = np.ascontiguousarray(x[b].reshape(C, N))
        in_maps.append({"x": xb, "xh": xb.astype(np.float16), **common})
    res = run_bass_kernel_spmd(nc, in_maps, list(range(N_CORES)))
    LAST_RESULTS = res
    out = np.stack([res.results[b]["out"].reshape(C, HH, WW) for b in range(B)])
    return out.astype(np.float32)


# revision 25
# speedup vs baseline: 1.4610x; 1.0332x over previous
"""Non-local block (self-attention over 64x64 spatial map) on 8 NeuronCores.

Sharding: data-parallel over batch (B=8 -> 1 image per core). Each core runs
the full N=4096 attention for its image; no collectives.

Per-core layout strategy:
  - theta_x/phi_x stored [O=96, N=4096] fp16; scores computed TRANSPOSED
    (S^T[m, q] chunks) so softmax denominators come out of the PE itself via
    an appended ones-column on the PV lhsT.
  - exp(S - 45) on ScalarE in [128, 1024] chunks (constant shift cancels
    exactly in softmax; bigger chunks amortize the ACT access bubble).
  - PV: lhsT = [g^T | ones] [128m, 97], rhs = expS^T [128m, 512q]
    -> y_u [97, 512] accumulated over 32 m-chunks; row 96 = softmax sums.
  - normalization commutes with the linear out-projection: 1/l is broadcast
    across partitions with a single-pass float32r PE outer product, applied
    after W^T y_u, off the critical path (tails software-pipelined one
    q-super behind the matmul stream).
"""

import numpy as np
import ml_dtypes

B, C, O = 8, 192, 96
HH, WW = 64, 64
N = HH * WW           # 4096
NQ = 8                # q-supers of 512
QS = 512
NMC = N // 128        # 32 m-chunks
N_CORES = 8

_CACHE = {}


def _build():
    from contextlib import ExitStack
    import concourse.tile as tile
    from concourse import bacc, mybir

    dt = mybir.dt
    AF = mybir.ActivationFunctionType

    nc = bacc.Bacc("TRN2", target_bir_lowering=False, debug=False,
                   num_devices=N_CORES)

    x_d = nc.dram_tensor("x", [C, N], dt.float32, kind="ExternalInput").ap()
    xh_d = nc.dram_tensor("xh", [C, N], dt.float16, kind="ExternalInput").ap()
    wt_d = {}
    b_d = {}
    for p in ("theta", "phi", "g"):
        wt_d[p] = nc.dram_tensor(f"wt_{p}", [C, O], dt.float16,
                                 kind="ExternalInput").ap()
        b_d[p] = nc.dram_tensor(f"b_{p}", [O, 1], dt.float32,
                                kind="ExternalInput").ap()
    wWT_d = nc.dram_tensor("w_WT", [O, C], dt.bfloat16, kind="ExternalInput").ap()
    bW_d = nc.dram_tensor("b_W", [C, 1], dt.float32, kind="ExternalInput").ap()
    out_d = nc.dram_tensor("out", [C, N], dt.float32, kind="ExternalOutput").ap()

    with tile.TileContext(nc) as tc:
        with ExitStack() as ctx:
            # ---------------- SBUF pools ----------------
            consts = ctx.enter_context(tc.tile_pool(name="consts", bufs=1))
            xpool = ctx.enter_context(tc.tile_pool(name="x", bufs=1))
            acts = ctx.enter_context(tc.tile_pool(name="acts", bufs=1))
            expp = ctx.enter_context(tc.tile_pool(name="exp", bufs=1))
            ypool = ctx.enter_context(tc.tile_pool(name="y", bufs=3))
            outp = ctx.enter_context(tc.tile_pool(name="outsb", bufs=3))
            # ---------------- PSUM pools (shared by all phases) ----------
            ps_qk = ctx.enter_context(
                tc.tile_pool(name="ps_qk", bufs=3, space="PSUM"))
            ps_pv = ctx.enter_context(
                tc.tile_pool(name="ps_pv", bufs=2, space="PSUM"))

            wt = {}
            bias = {}
            for p in ("theta", "phi", "g"):
                wt[p] = consts.tile([96, 2 * O], dt.float16, tag=f"wt_{p}",
                                    name=f"wt_{p}")
                nc.gpsimd.dma_start(wt[p][:, 0:O], wt_d[p][0:96, :])
                nc.gpsimd.dma_start(wt[p][:, O:2 * O], wt_d[p][96:192, :])
                bias[p] = consts.tile([O, 1], dt.float32, tag=f"b_{p}",
                                      name=f"b_{p}")
                nc.gpsimd.dma_start(bias[p][:], b_d[p][:])
            wWT = consts.tile([O, C], dt.bfloat16, tag="wWT")
            nc.gpsimd.dma_start(wWT[:], wWT_d[:])
            bW = [consts.tile([96, 1], dt.float32, tag=f"bW{h}", name=f"bW{h}")
                  for h in (0, 1)]
            for h in (0, 1):
                nc.gpsimd.dma_start(bW[h][:], bW_d[96 * h:96 * h + 96, :])

            cneg45 = consts.tile([128, 1], dt.float32, tag="cneg45")
            nc.vector.memset(cneg45[:], -45.0)

            theta_sb = acts.tile([O, N], dt.float16, tag="theta")
            phi_sb = acts.tile([O, N], dt.float16, tag="phi")
            gt_ones = acts.tile([128, 97 * NMC], dt.bfloat16, tag="gt")
            nc.vector.memset(gt_ones[:], 1.0)
            expS = expp.tile([128, NMC * QS], dt.bfloat16, tag="expS")

            # x halves: fp16 for matmul (first: it gates everything),
            # fp32 for the residual
            xh = [xpool.tile([96, N], dt.float16, tag=f"xh{h}", name=f"xh{h}")
                  for h in (0, 1)]
            xf = [xpool.tile([96, N], dt.float32, tag=f"xf{h}", name=f"xf{h}")
                  for h in (0, 1)]
            for j in range(NQ):
                cs = slice(j * QS, (j + 1) * QS)
                for h in (0, 1):
                    nc.sync.dma_start(xh[h][:, cs], xh_d[96 * h:96 * h + 96, cs])
            for j in range(NQ):
                cs = slice(j * QS, (j + 1) * QS)
                for h in (0, 1):
                    nc.gpsimd.dma_start(xf[h][:, cs], x_d[96 * h:96 * h + 96, cs])
            for h in (0, 1):
                nc.vector.tensor_scalar_add(xf[h][:], xf[h][:], bW[h][:])

            # ---------------- P1 + P2, software-pipelined ----------------
            # window w streams QK(w)+exp(w) on ACT while PE also runs
            # PV(w-1), tail(w-2), and dribbles leftover projection work.
            # g^T is produced directly by the PE (lhsT = x chunk), its bias
            # is folded into b_W on the host.

            def emit_proj_mms(p, pr, k, h, state):
                if "ps" not in state:
                    state["ps"] = ps_qk.tile([128, 2 * QS], dt.float32,
                                             tag="qk", name=f"proj_{p}_{pr}")
                ps = state["ps"]
                cs = slice((2 * pr + k) * QS, (2 * pr + k + 1) * QS)
                nc.tensor.matmul(ps[0:O, k * QS:(k + 1) * QS],
                                 wt[p][:, 96 * h:96 * h + O],
                                 xh[h][:, cs],
                                 start=(h == 0), stop=(h == 1))
                if k == 1 and h == 1:
                    dst = {"phi": phi_sb, "theta": theta_sb}[p]
                    pcs = slice(pr * 2 * QS, (pr + 1) * 2 * QS)
                    nc.vector.tensor_scalar_add(dst[:, pcs], ps[0:O, :],
                                                bias[p][:])

            def emit_gt(k2):
                # gt chunks 2*k2, 2*k2+1 via lhsT = x chunk (direct g^T)
                ps = ps_qk.tile([128, 2 * QS], dt.float32, tag="qk",
                                name=f"gt_{k2}")
                for j in (0, 1):
                    mc = 2 * k2 + j
                    mcs = slice(128 * mc, 128 * mc + 128)
                    nc.tensor.matmul(ps[:, 512 * j:512 * j + 96],
                                     xh[0][:, mcs], wt["g"][:, 0:O],
                                     start=True, stop=False)
                    nc.tensor.matmul(ps[:, 512 * j:512 * j + 96],
                                     xh[1][:, mcs], wt["g"][:, O:2 * O],
                                     start=False, stop=True)
                a = 2 * k2
                dstv = gt_ones[:, 97 * a:97 * (a + 2)].rearrange(
                    "p (c w) -> p c w", w=97)[:, :, 0:96]
                srcv = ps[:, 0:1024].rearrange(
                    "p (c w) -> p c w", w=512)[:, :, 0:96]
                nc.vector.tensor_copy(dstv, srcv)

            def proj_items(p, pr):
                state = {}
                return [lambda k=k, h=h: emit_proj_mms(p, pr, k, h, state)
                        for k in (0, 1) for h in (0, 1)]

            def emit_qk_exp(qs, pr):
                ps = ps_qk.tile([128, 2 * QS], dt.float32, tag="qk",
                                name=f"qk_{qs}_{pr}")
                qcols = slice(qs * QS, (qs + 1) * QS)
                for k in (0, 1):
                    mc = 2 * pr + k
                    nc.tensor.matmul(
                        ps[:, k * QS:(k + 1) * QS],
                        phi_sb[:, 128 * mc:128 * mc + 128],
                        theta_sb[:, qcols], start=True, stop=True)
                nc.scalar.activation(
                    expS[:, 1024 * pr:1024 * pr + 1024], ps[:],
                    AF.Exp, bias=cneg45[:])

            def emit_pv(qs, mc, ypsum):
                nc.tensor.matmul(
                    ypsum[:], gt_ones[:, 97 * mc:97 * mc + 97],
                    expS[:, 512 * mc:512 * mc + 512],
                    start=(mc == 0), stop=(mc == NMC - 1))

            tails = {}

            def tail_step(qs, pr, ypsum):
                if pr == 1:
                    y_bf = ypool.tile([O, QS], dt.bfloat16, tag="ybf",
                                      name=f"ybf_{qs}")
                    nc.vector.tensor_copy(y_bf[:], ypsum[0:96, :])
                    linv_sb = ypool.tile([1, QS], dt.float32, tag="linv",
                                         name=f"linv_{qs}")
                    nc.vector.reciprocal(linv_sb[:], ypsum[96:97, :])
                    li_sb = ypool.tile([O, QS], dt.float32, tag="lisb",
                                       name=f"lisb_{qs}")
                    nc.gpsimd.partition_broadcast(li_sb[:], linv_sb[:])
                    tails[qs] = (y_bf, li_sb, None)
                elif pr in (2, 3) and qs in tails:
                    h = pr - 2
                    y_bf, li_sb = tails[qs][0], tails[qs][1]
                    qcols = slice(qs * QS, (qs + 1) * QS)
                    if h == 0:
                        tails[qs] = (y_bf, li_sb,
                                     ps_qk.tile([128, 2 * QS], dt.float32,
                                                tag="qk", name=f"pso_{qs}"))

                    pso = tails[qs][2]
                    nc.tensor.matmul(pso[0:96, h * QS:(h + 1) * QS],
                                     wWT[:, 96 * h:96 * h + 96],
                                     y_bf[:], start=True, stop=True)
                    ob = outp.tile([96, QS], dt.float32, tag="ob",
                                   name=f"ob_{qs}_{h}")
                    nc.vector.tensor_mul(ob[:],
                                         pso[0:96, h * QS:(h + 1) * QS],
                                         li_sb[:])
                    nc.vector.tensor_add(ob[:], ob[:], xf[h][:, qcols])
                    nc.sync.dma_start(out_d[96 * h:96 * h + 96, qcols], ob[:])

            p0 = {}
            t0s = {}
            for k in (0, 1):
                for h in (0, 1):
                    emit_proj_mms("phi", 0, k, h, p0)
            for k in (0, 1):
                for h in (0, 1):
                    emit_proj_mms("theta", 0, k, h, t0s)

            work = []
            for pr in (1, 2, 3):
                work += proj_items("phi", pr)
            work += [lambda k2=k2: emit_gt(k2) for k2 in range(NMC // 2)]
            for pr in (1, 2, 3):
                work += proj_items("theta", pr)

            ypsums = {}
            for w in range(NQ):
                if w >= 1:
                    ypsums[w - 1] = ps_pv.tile([97, QS], dt.float32, tag="pv",
                                               name=f"pv_{w - 1}")
                for pr in range(NMC // 2):
                    if not (w == 0 and pr == 0) and work:
                        work.pop(0)()
                    if w >= 1:
                        for k in (0, 1):
                            emit_pv(w - 1, 2 * pr + k, ypsums[w - 1])
                    emit_qk_exp(w, pr)
                    if w >= 2:
                        tail_step(w - 2, pr, ypsums[w - 2])
            # epilogue: PV(7), tail(6), tail(7)
            ypsums[NQ - 1] = ps_pv.tile([97, QS], dt.float32, tag="pv",
                                        name=f"pv_{NQ - 1}")
            for pr in range(NMC // 2):
                for k in (0, 1):
                    emit_pv(NQ - 1, 2 * pr + k, ypsums[NQ - 1])
                tail_step(NQ - 2, pr, ypsums[NQ - 2])
            for pr in (1, 2, 3):
                tail_step(NQ - 1, pr, ypsums[NQ - 1])
            while work:
                work.pop(0)()

    nc.compile()
    return nc


def _get_nc():
    if "nc" not in _CACHE:
        _CACHE["nc"] = _build()
    return _CACHE["nc"]


LAST_RESULTS = None


def kernel(x, g_w, g_b, theta_w, theta_b, phi_w, phi_b, W_w, W_b):
    global LAST_RESULTS
    from concourse.bass_utils import run_bass_kernel_spmd

    nc = _get_nc()

    x = np.asarray(x, dtype=np.float32)
    common = {
        "wt_theta": np.ascontiguousarray(np.asarray(theta_w).T).astype(np.float16),
        "wt_phi": np.ascontiguousarray(np.asarray(phi_w).T).astype(np.float16),
        "wt_g": np.ascontiguousarray(np.asarray(g_w).T).astype(np.float16),
        "w_WT": np.ascontiguousarray(np.asarray(W_w).T).astype(ml_dtypes.bfloat16),
        "b_theta": np.asarray(theta_b, dtype=np.float32).reshape(O, 1),
        "b_phi": np.asarray(phi_b, dtype=np.float32).reshape(O, 1),
        "b_g": np.asarray(g_b, dtype=np.float32).reshape(O, 1),
        "b_W": (np.asarray(W_b, dtype=np.float32)
                + np.asarray(W_w, dtype=np.float32)
                @ np.asarray(g_b, dtype=np.float32)).reshape(C, 1),
    }
    in_maps = []
    for b in range(B):
        xb = np.ascontiguousarray(x[b].reshape(C, N))
        in_maps.append({"x": xb, "xh": xb.astype(np.float16), **common})
    res = run_bass_kernel_spmd(nc, in_maps, list(range(N_CORES)))
    LAST_RESULTS = res
    out = np.stack([res.results[b]["out"].reshape(C, HH, WW) for b in range(B)])
    return out.astype(np.float32)
